# revision 34
# baseline (speedup 1.0000x reference)
"""Trainium2 Bass kernel for CustomPoseMixtureVAE (moe_routing).

Strategy: data-parallel over batch across 8 NeuronCores (256 rows/core),
all weights replicated, no collectives. Activations kept feature-major
[feat, batch] on-chip so every linear is a single PSUM-accumulated GEMM
chain with the (host-pretransposed) weight as the stationary operand.

The expert mixture  out = einsum('be,bi,eio->bo', coeff, inp, W)  is
computed as ONE GEMM over K = E*in using per-expert coefficient-scaled
inputs stacked along K:  out[b,o] = sum_{e,i} (coeff[b,e]*inp[b,i]) W[e,i,o].
Coefficient rows are broadcast across partitions with one-hot selector
matmuls on the PE; scaling is elementwise on DVE. The mixed bias
(coeff @ b_e) is folded in as an extra K-tile whose rhs is coeffT and
whose lhsT is the bias matrix.

ELU(x) = max(x, min(exp(x),1) - 1)   (exp monotonic => exp(min(x,0)) =
min(exp(x),1)); exp on ScalarE, the rest on Vector/GpSimd engines.
"""

import numpy as np
from contextlib import ExitStack

import concourse.bass as bass
import concourse.bacc as bacc
import concourse.tile as tile
import concourse.mybir as mybir
from concourse.bass_utils import run_bass_kernel_spmd

F16 = mybir.dt.float16
F32 = mybir.dt.float32
AOP = mybir.AluOpType
AF = mybir.ActivationFunctionType

B = 2048
NCORES = 8
BC = B // NCORES          # 256 batch rows per core
F = 267                   # frame size
L = 32                    # latent
H = 256                   # hidden
E = 8                     # experts
G = 64                    # gate hidden
IN0 = L + F               # 299
IN1 = L + H               # 288

LAST_RESULTS = None       # BassKernelResults of the most recent run
_CACHE = {}


def _eps42():
    """eps = jax.random.normal(key(42), (B, L)) exactly as the reference
    computes it, on the default jax backend (PRNG lowerings differ between
    backends, so we must mirror the reference's code path, not hardcode)."""
    if "eps" not in _CACHE:
        import jax
        import jax.numpy as jnp

        _CACHE["eps"] = np.asarray(
            jax.random.normal(jax.random.key(42), (B, L), jnp.float32)
        )
    return _CACHE["eps"]


def _pad_rows(a, rows):
    out = np.zeros((rows, a.shape[1]), a.dtype)
    out[: a.shape[0]] = a
    return out


def _bcast_ap(ap2d, reps):
    """[P, N] AP -> [P, reps, N] AP with a step-0 middle dim (operand bcast)."""
    return bass.AP(
        tensor=ap2d.tensor,
        offset=ap2d.offset,
        ap=[ap2d.ap[0], [0, reps], ap2d.ap[1]],
    )


def _build_program():
    nc = bacc.Bacc("TRN2")
    d = {}

    def din(name, shape, dt=F16):
        d[name] = nc.dram_tensor(name, shape, dt, kind="ExternalInput").ap()

    def dout(name, shape, dt=F32):
        d[name] = nc.dram_tensor(name, shape, dt, kind="ExternalOutput").ap()

    # per-core activations ([feat, batch] fp16; xo/co carry a trailing ones row)
    din("xo", [268, BC])
    din("co", [268, BC])
    din("epsT", [L, BC])
    # replicated weights (host-restacked, see kernel())
    din("sel", [8, 9, 128])
    din("ident", [128, 128], F32)
    din("fc1w", [6 * 128, 256])
    din("fc2w", [5 * 128, 256])
    din("mvw", [5 * 128, 64])
    din("g0w", [4 * 128, 64])
    din("g1w", [128, 64])
    din("g1wb", [1, 64])
    din("g2w", [128, 8])
    din("g2wb", [1, 8])
    din("w0st", [21 * 128, 256])
    din("w1st", [19 * 128, 256])
    din("w2st", [19 * 128, 267])
    dout("outT", [F, BC])
    dout("mvT", [2 * L, BC])

    with tile.TileContext(nc) as tc, ExitStack() as ctx:
        sb = ctx.enter_context(tc.tile_pool(name="sb", bufs=1))
        ps_big = ctx.enter_context(tc.tile_pool(name="ps_big", bufs=2, space="PSUM"))
        ps_sm = ctx.enter_context(tc.tile_pool(name="ps_sm", bufs=3, space="PSUM"))
        ps_bc = ctx.enter_context(tc.tile_pool(name="ps_bc", bufs=2, space="PSUM"))

        def sbt(tag, shape, dt=F16):
            return sb.tile(shape, dt, tag=tag, name=tag)

        # ---- input / weight DMAs -------------------------------------
        def dma_tiled(dst, src, nk, cols, eng=None):
            """DRAM [nk*128, cols] -> SBUF [128, nk, cols]."""
            eng = eng or nc.sync
            eng.dma_start(
                dst[:, 0:nk, :],
                src[0 : nk * 128, :].rearrange("(k p) m -> p k m", p=128),
            )

        # Spread DMAs over all five engines' queues so they pull in parallel,
        # first-needed tensors first on each queue.
        engs = [nc.sync, nc.scalar, nc.gpsimd]

        xo = sbt("xo", [128, 3, BC])
        nc.sync.dma_start(xo[:, 0:2, :], d["xo"][0:256, :].rearrange("(k p) n -> p k n", p=128))
        nc.sync.dma_start(xo[0:12, 2, :], d["xo"][256:268, :])
        co = sbt("co", [128, 3, BC])
        nc.scalar.dma_start(co[:, 0:2, :], d["co"][0:256, :].rearrange("(k p) n -> p k n", p=128))
        nc.scalar.dma_start(co[0:12, 2, :], d["co"][256:268, :])
        fc1w = sbt("fc1w", [128, 6, 256])
        nc.sync.dma_start(fc1w[:, 0:3, :], d["fc1w"][0:384, :].rearrange("(k p) m -> p k m", p=128))
        nc.scalar.dma_start(fc1w[:, 3:6, :], d["fc1w"][384:768, :].rearrange("(k p) m -> p k m", p=128))
        eps = sbt("eps", [L, BC])
        nc.sync.dma_start(eps[:], d["epsT"][:])
        sel = sbt("sel", [8, 9, 128])
        nc.scalar.dma_start(sel[:], d["sel"][:])
        ident = sbt("ident", [128, 128], F32)
        nc.sync.dma_start(ident[:], d["ident"][:])

        fc2w = sbt("fc2w", [128, 5, 256])
        dma_tiled(fc2w, d["fc2w"], 5, 256, eng=nc.sync)
        mvw = sbt("mvw", [128, 5, 64])
        dma_tiled(mvw, d["mvw"], 5, 64, eng=nc.scalar)
        g0w = sbt("g0w", [128, 4, 64])
        dma_tiled(g0w, d["g0w"], 4, 64, eng=nc.scalar)
        g1w = sbt("g1w", [128, 64])
        nc.sync.dma_start(g1w[:], d["g1w"][:])
        g1wb = sbt("g1wb", [1, 64])
        nc.sync.dma_start(g1wb[:], d["g1wb"][:])
        g2w = sbt("g2w", [128, 8])
        nc.scalar.dma_start(g2w[:], d["g2w"][:])
        g2wb = sbt("g2wb", [1, 8])
        nc.scalar.dma_start(g2wb[:], d["g2wb"][:])

        # decoder weights: chunked, alternating between the two HWDGE queues
        # (keep SWDGE/gpsimd for the small early tensors only — its per-DMA
        # overhead and queue drain are expensive)
        w0 = sbt("w0", [128, 21, 256])
        w1 = sbt("w1", [128, 19, 256])
        w2 = sbt("w2", [128, 19, 267])
        hw = [nc.sync, nc.scalar]
        qi = 0
        for (wt, src, nk) in ((w0, d["w0st"], 21), (w1, d["w1st"], 19), (w2, d["w2st"], 19)):
            for lo in range(0, nk, 4):
                hi = min(lo + 4, nk)
                hw[qi % 2].dma_start(
                    wt[:, lo:hi, :],
                    src[lo * 128 : hi * 128, :].rearrange("(k p) m -> p k m", p=128),
                )
                qi += 1

        # L0 c-tail scaled tiles: zero the 21-row pads once, up front
        s0t = sbt("s0t", [128, 2, BC])
        nc.vector.memset(s0t[:], 0.0)

        # ---- helpers -------------------------------------------------
        def elu(pp, width, out_ap, tagbase):
            """ELU(x) = relu(x) + (min(exp(x),1) - 1), x = psum pp [P, width]."""
            p = pp.shape[0]
            ex = sbt(tagbase + "_e", [p, width])
            nc.scalar.activation(ex[:], pp, AF.Exp)
            tt = sbt(tagbase + "_t", [p, width])
            nc.vector.tensor_scalar(tt[:], ex[:], 1.0, 1.0, AOP.min, AOP.subtract)
            nc.vector.scalar_tensor_tensor(out_ap, pp, 0.0, tt[:], AOP.max, AOP.add)

        # ---- encoder -------------------------------------------------
        with nc.named_scope("enc1"):
            ph1 = ps_big.tile([128, 512], F32, tag="pbig", name="pbig")
            enc1_rhs = [
                (xo[:, 0, :], slice(0, 128)),
                (xo[:, 1, :], slice(0, 128)),
                (xo[0:12, 2, :], slice(0, 12)),
                (co[:, 0, :], slice(0, 128)),
                (co[:, 1, :], slice(0, 128)),
                (co[0:12, 2, :], slice(0, 12)),
            ]
            for m in range(2):
                n = len(enc1_rhs)
                for i, (rh, krows) in enumerate(enc1_rhs):
                    nc.tensor.matmul(
                        ph1[:, m * BC : (m + 1) * BC],
                        fc1w[krows, i, m * 128 : (m + 1) * 128],
                        rh,
                        start=(i == 0),
                        stop=(i == n - 1),
                    )
            h1 = sbt("h1", [128, 2, BC])
            elu(ph1[:], 512, h1[:].rearrange("p k n -> p (k n)"), "eh1")

        with nc.named_scope("enc2"):
            ph2 = ps_big.tile([128, 512], F32, tag="pbig", name="pbig")
            enc2_rhs = [
                (xo[:, 0, :], slice(0, 128)),
                (xo[:, 1, :], slice(0, 128)),
                (xo[0:12, 2, :], slice(0, 12)),
                (h1[:, 0, :], slice(0, 128)),
                (h1[:, 1, :], slice(0, 128)),
            ]
            for m in range(2):
                n = len(enc2_rhs)
                for i, (rh, krows) in enumerate(enc2_rhs):
                    nc.tensor.matmul(
                        ph2[:, m * BC : (m + 1) * BC],
                        fc2w[krows, i, m * 128 : (m + 1) * 128],
                        rh,
                        start=(i == 0),
                        stop=(i == n - 1),
                    )
            h2 = sbt("h2", [128, 2, BC])
            elu(ph2[:], 512, h2[:].rearrange("p k n -> p (k n)"), "eh2")

        with nc.named_scope("muvar"):
            pmv = ps_sm.tile([64, BC], F32, tag="psm", name="psm")
            mv_rhs = [
                (xo[:, 0, :], slice(0, 128)),
                (xo[:, 1, :], slice(0, 128)),
                (xo[0:12, 2, :], slice(0, 12)),
                (h2[:, 0, :], slice(0, 128)),
                (h2[:, 1, :], slice(0, 128)),
            ]
            n = len(mv_rhs)
            for i, (rh, krows) in enumerate(mv_rhs):
                nc.tensor.matmul(pmv[:], mvw[krows, i, 0:64], rh, start=(i == 0), stop=(i == n - 1))
            mvf = sbt("mvf", [64, BC], F32)
            nc.scalar.copy(mvf[:], pmv[:])                 # fp32 mu/logvar out
            nc.sync.dma_start(d["mvT"][:], mvf[:])
            # z = mu + eps * exp(0.5*logvar)
            stdt = sbt("stdt", [L, BC])
            nc.scalar.activation(stdt[:], pmv[32:64, :], AF.Exp, scale=0.5)
            zt = sbt("zt", [L, BC])
            nc.vector.tensor_mul(zt[:], eps[:], stdt[:])
            z = sbt("z", [L, BC])
            nc.vector.tensor_add(z[:], zt[:], pmv[0:32, :])

        # ---- gate ----------------------------------------------------
        # Gate ELUs are folded into the next matmul:  W@elu(x) = W@relu(x)
        # + W@t with t = min(exp(x),1)-1, so the 64-row relu/t halves stack
        # into one 128-row K-tile (weights duplicated host-side) and the DVE
        # max-combine drops off the serial chain.
        with nc.named_scope("gate"):
            onesr = sbt("onesr", [1, BC])
            nc.vector.memset(onesr[:], 1.0)
            pg0 = ps_sm.tile([64, BC], F32, tag="psm", name="psm")
            # z-free tiles first so the PE can start before z is ready
            g0_rhs = [
                (co[:, 0, :], slice(0, 128)),
                (co[:, 1, :], slice(0, 128)),
                (co[0:12, 2, :], slice(0, 12)),
                (z[:], slice(0, 32)),
            ]
            n = len(g0_rhs)
            for i, (rh, krows) in enumerate(g0_rhs):
                nc.tensor.matmul(pg0[:], g0w[krows, i, 0:64], rh, start=(i == 0), stop=(i == n - 1))

            def gate_relu_t(pp, tagbase):
                ga = sbt(tagbase + "_a", [128, BC])
                ex = sbt(tagbase + "_e", [64, BC])
                nc.scalar.activation(ex[:], pp, AF.Exp)
                nc.scalar.activation(ga[0:64, :], pp, AF.Relu)
                nc.vector.tensor_scalar(ga[64:128, :], ex[:], 1.0, 1.0, AOP.min, AOP.subtract)
                return ga

            g1in = gate_relu_t(pg0[:], "eg0")
            pg1 = ps_sm.tile([64, BC], F32, tag="psm", name="psm")
            nc.tensor.matmul(pg1[:], g1w[:, 0:64], g1in[:], start=True, stop=False)
            nc.tensor.matmul(pg1[:], g1wb[0:1, 0:64], onesr[:], start=False, stop=True)

            g2in = gate_relu_t(pg1[:], "eg1")
            plg = ps_sm.tile([8, BC], F32, tag="psm", name="psm")
            nc.tensor.matmul(plg[:], g2w[:, 0:8], g2in[:], start=True, stop=False)
            nc.tensor.matmul(plg[:], g2wb[0:1, 0:8], onesr[:], start=False, stop=True)

            # softmax over the 8 experts (partition dim) without max-sub.
            # The 1/sum is computed batch-on-partitions so the DVE iterative
            # divide runs 2 elems/lane x 128 lanes instead of 256 on one lane.
            expE = sbt("expE", [8, BC])
            nc.scalar.activation(expE[:], plg[:], AF.Exp)
            pst = ps_sm.tile([128, 2], F32, tag="psm", name="psm")
            nc.tensor.matmul(pst[:, 0:1], expE[0:8, 0:128], sel[0:8, 8, 0:1], start=True, stop=True)
            nc.tensor.matmul(pst[:, 1:2], expE[0:8, 128:256], sel[0:8, 8, 0:1], start=True, stop=True)
            rst = sbt("rst", [128, 2], F32)
            with nc.allow_low_precision(reason="softmax denominators are well-conditioned"):
                nc.vector.reciprocal(rst[:], pst[:])
            precS = ps_sm.tile([1, BC], F32, tag="psm", name="psm")
            nc.tensor.matmul(precS[0:1, 0:128], rst[:, 0:1], ident[:], start=True, stop=True)
            nc.tensor.matmul(precS[0:1, 128:256], rst[:, 1:2], ident[:], start=True, stop=True)
            recS = sbt("recS", [1, BC])
            nc.scalar.copy(recS[:], precS[:])
            pr8 = ps_sm.tile([8, BC], F32, tag="psm", name="psm")
            nc.tensor.matmul(pr8[:], sel[0:1, 8, 0:8], recS[:], start=True, stop=True)
            coeffT = sbt("coeffT", [8, BC])
            nc.vector.tensor_mul(coeffT[:], expE[:], pr8[:])

            # broadcast each coeff row to 128 partitions: sel_e.T @ coeffT
            bcastC = sbt("bcastC", [128, 8, BC])
            for i in range(4):
                pbc = ps_bc.tile([128, 512], F32, tag="pbc", name="pbc")
                nc.tensor.matmul(pbc[:, 0:BC], sel[:, 2 * i, :], coeffT[:], start=True, stop=True)
                nc.tensor.matmul(pbc[:, BC:512], sel[:, 2 * i + 1, :], coeffT[:], start=True, stop=True)
                cpeng = nc.vector if i % 2 == 0 else nc.scalar
                if cpeng is nc.vector:
                    nc.vector.tensor_copy(bcastC[:, 2 * i : 2 * i + 2, :].rearrange("p k n -> p (k n)"), pbc[:])
                else:
                    nc.scalar.copy(bcastC[:, 2 * i : 2 * i + 2, :].rearrange("p k n -> p (k n)"), pbc[:])

        # ---- decoder -------------------------------------------------
        # z-part of the scaled stack (shared by all 3 layers)
        with nc.named_scope("zstack"):
            zs = sbt("zs", [128, 2, BC])
            for e in range(E):
                nc.vector.tensor_tensor(
                    zs[32 * (e % 4) : 32 * (e % 4) + 32, e // 4, :],
                    z[:],
                    bcastC[0:32, e, :],
                    AOP.mult,
                )

        def scaled_stack(name, src3, nsub):
            """Per-expert coefficient-scaled copies of src3 [128, nsub, BC]."""
            st = sbt(name, [128, E * nsub, BC])
            for e in range(E):
                nc.vector.tensor_tensor(
                    st[:, e * nsub : (e + 1) * nsub, :],
                    src3,
                    _bcast_ap(bcastC[:, e, :], nsub),
                    AOP.mult,
                )
            return st

        def decoder_layer(name, wt, stack_tiles, psum_w, mtiles, out_elu):
            """stack_tiles: list of (rhs_ap, krows) aligned with wt subtiles."""
            with nc.named_scope(name):
                pl = ps_big.tile([128, 512], F32, tag="pbig", name="pbig") if psum_w == 512 else None
                outs = []
                for mi, (mlo, mhi) in enumerate(mtiles):
                    if mhi - mlo == 128 and pl is not None:
                        pap = pl[:, mi * BC : (mi + 1) * BC]
                    else:
                        ptail = ps_sm.tile([mhi - mlo, BC], F32, tag="psm", name="psm")
                        pap = ptail[:]
                        outs.append(ptail)
                    n = len(stack_tiles)
                    for i, (rh, krows) in enumerate(stack_tiles):
                        nc.tensor.matmul(
                            pap,
                            wt[krows, i, mlo:mhi],
                            rh,
                            start=(i == 0),
                            stop=(i == n - 1),
                        )
                if out_elu is not None:
                    elu(pl[:], 512, out_elu[:].rearrange("p k n -> p (k n)"), "e" + name)
                return pl, outs

        with nc.named_scope("l0scale"):
            s0 = scaled_stack("s0", co[:, 0:2, :], 2)
            # c tail rows (11 per expert, padded to 32-partition blocks: compute
            # engines require 32-aligned partition bases)
            for e in range(E):
                nc.vector.tensor_tensor(
                    s0t[32 * (e % 4) : 32 * (e % 4) + 11, e // 4, :],
                    co[0:11, 2, :],
                    bcastC[0:11, e, :],
                    AOP.mult,
                )

        l0_tiles = [(s0[:, j, :], slice(0, 128)) for j in range(16)]
        l0_tiles += [(s0t[:, 0, :], slice(0, 128)), (s0t[:, 1, :], slice(0, 128))]
        l0_tiles += [(zs[:, 0, :], slice(0, 128)), (zs[:, 1, :], slice(0, 128)), (coeffT[:], slice(0, 8))]
        l0o = sbt("l0o", [128, 2, BC])
        decoder_layer("l0", w0, l0_tiles, 512, [(0, 128), (128, 256)], l0o)

        with nc.named_scope("l1scale"):
            s1 = scaled_stack("s1", l0o[:, 0:2, :], 2)
        l1_tiles = [(s1[:, j, :], slice(0, 128)) for j in range(16)]
        l1_tiles += [(zs[:, 0, :], slice(0, 128)), (zs[:, 1, :], slice(0, 128)), (coeffT[:], slice(0, 8))]
        l1o = sbt("l1o", [128, 2, BC])
        decoder_layer("l1", w1, l1_tiles, 512, [(0, 128), (128, 256)], l1o)

        with nc.named_scope("l2scale"):
            s2 = scaled_stack("s2", l1o[:, 0:2, :], 2)
        l2_tiles = [(s2[:, j, :], slice(0, 128)) for j in range(16)]
        l2_tiles += [(zs[:, 0, :], slice(0, 128)), (zs[:, 1, :], slice(0, 128)), (coeffT[:], slice(0, 8))]
        pl2, tails = decoder_layer(
            "l2", w2, l2_tiles, 512, [(0, 128), (128, 256), (256, 267)], None
        )

        with nc.named_scope("out"):
            out0 = sbt("out0", [128, 512], F32)
            nc.scalar.copy(out0[:, 0:BC], pl2[:, 0:BC])
            nc.sync.dma_start(d["outT"][0:128, :], out0[:, 0:BC])
            nc.vector.tensor_copy(out0[:, BC:512], pl2[:, BC:512])
            nc.scalar.dma_start(d["outT"][128:256, :], out0[:, BC:512])
            out2 = sbt("out2", [11, BC], F32)
            nc.scalar.copy(out2[:], tails[0][:])
            nc.sync.dma_start(d["outT"][256:267, :], out2[:])

    nc.compile()
    return nc


def _host_weights(i):
    """Restack/transpose/cast all weights for the device layout."""
    f16 = np.float16

    def t(a):
        return np.asarray(a, np.float32).T  # [in, out]

    W1t = t(i["fc1_w"])  # [534, 256]
    fc1 = np.concatenate(
        [
            W1t[0:128],
            W1t[128:256],
            _pad_rows(np.concatenate([W1t[256:267], i["fc1_b"][None, :]], 0), 128),
            W1t[267:395],
            W1t[395:523],
            _pad_rows(W1t[523:534], 128),
        ],
        0,
    )
    W2t = t(i["fc2_w"])  # [523, 256]
    fc2 = np.concatenate(
        [
            W2t[0:128],
            W2t[128:256],
            _pad_rows(np.concatenate([W2t[256:267], i["fc2_b"][None, :]], 0), 128),
            W2t[267:395],
            W2t[395:523],
        ],
        0,
    )
    Wmv = np.concatenate([t(i["mu_w"]), t(i["lv_w"])], 1)  # [523, 64]
    bmv = np.concatenate([i["mu_b"], i["lv_b"]])[None, :]
    mv = np.concatenate(
        [
            Wmv[0:128],
            Wmv[128:256],
            _pad_rows(np.concatenate([Wmv[256:267], bmv], 0), 128),
            Wmv[267:395],
            Wmv[395:523],
        ],
        0,
    )
    G0 = t(i["g0_w"])  # [299, 64]
    g0 = np.concatenate(
        [
            G0[32:160],
            G0[160:288],
            _pad_rows(np.concatenate([G0[288:299], i["g0_b"][None, :]], 0), 128),
            _pad_rows(G0[0:32], 128),
        ],
        0,
    )
    g1 = np.concatenate([t(i["g1_w"]), t(i["g1_w"])], 0)  # relu-half + t-half
    g2 = np.concatenate([t(i["g2_w"]), t(i["g2_w"])], 0)

    def dec_stack(w, b):
        w = np.asarray(w, np.float32)  # [E, in, out]
        parts = []
        for e in range(E):
            parts.append(w[e, 32:160])
            parts.append(w[e, 160:288])
        if w.shape[1] == IN0:  # layer 0: c tail rows, 32-row block per expert
            for g in range(2):
                parts.append(
                    np.concatenate(
                        [_pad_rows(w[e, 288:299], 32) for e in range(4 * g, 4 * g + 4)], 0
                    )
                )
        parts.append(np.concatenate([w[e, 0:32] for e in range(4)], 0))
        parts.append(np.concatenate([w[e, 0:32] for e in range(4, 8)], 0))
        parts.append(_pad_rows(np.asarray(b, np.float32), 128))
        return np.concatenate(parts, 0)

    w0st = dec_stack(i["w0"], i["b0"])
    w1st = dec_stack(i["w1"], i["b1"])
    w2st = dec_stack(i["w2"], i["b2"])

    sel = np.zeros((8, 9, 128), np.float32)
    for e in range(E):
        sel[e, e, :] = 1.0
    sel[:, 8, :] = 1.0

    return {
        "ident": np.eye(128, dtype=np.float32),
        "sel": sel.astype(f16),
        "fc1w": fc1.astype(f16),
        "fc2w": fc2.astype(f16),
        "mvw": mv.astype(f16),
        "g0w": g0.astype(f16),
        "g1w": g1.astype(f16),
        "g1wb": np.asarray(i["g1_b"], np.float32)[None, :].astype(f16),
        "g2w": g2.astype(f16),
        "g2wb": np.asarray(i["g2_b"], np.float32)[None, :].astype(f16),
        "w0st": w0st.astype(f16),
        "w1st": w1st.astype(f16),
        "w2st": w2st.astype(f16),
    }


def kernel(**inputs):
    global LAST_RESULTS
    if "nc" not in _CACHE:
        _CACHE["nc"] = _build_program()
    nc = _CACHE["nc"]

    i = {k: np.asarray(v) for k, v in inputs.items()}
    eps = _eps42()
    wmap = _host_weights(i)

    ones = np.ones((1, B), np.float32)
    xo_full = np.concatenate([np.asarray(i["x"], np.float32).T, ones], 0).astype(np.float16)
    co_full = np.concatenate([np.asarray(i["c"], np.float32).T, ones], 0).astype(np.float16)
    epsT = eps.T.astype(np.float16)

    in_maps = []
    for ci in range(NCORES):
        s = slice(ci * BC, (ci + 1) * BC)
        m = dict(wmap)
        m["xo"] = np.ascontiguousarray(xo_full[:, s])
        m["co"] = np.ascontiguousarray(co_full[:, s])
        m["epsT"] = np.ascontiguousarray(epsT[:, s])
        in_maps.append(m)

    res = run_bass_kernel_spmd(nc, in_maps, core_ids=list(range(NCORES)))
    LAST_RESULTS = res

    out = np.empty((B, F), np.float32)
    mu = np.empty((B, L), np.float32)
    lv = np.empty((B, L), np.float32)
    for ci in range(NCORES):
        s = slice(ci * BC, (ci + 1) * BC)
        r = res.results[ci]
        out[s] = r["outT"].T
        mu[s] = r["mvT"][0:L].T
        lv[s] = r["mvT"][L : 2 * L].T
    return out, mu, lv


# revision 35
# speedup vs baseline: 1.0295x; 1.0295x over previous
"""Trainium2 Bass kernel for CustomPoseMixtureVAE (moe_routing).

Strategy: data-parallel over batch across 8 NeuronCores (256 rows/core),
all weights replicated, no collectives. Activations kept feature-major
[feat, batch] on-chip so every linear is a single PSUM-accumulated GEMM
chain with the (host-pretransposed) weight as the stationary operand.

The expert mixture  out = einsum('be,bi,eio->bo', coeff, inp, W)  is
computed as ONE GEMM over K = E*in using per-expert coefficient-scaled
inputs stacked along K:  out[b,o] = sum_{e,i} (coeff[b,e]*inp[b,i]) W[e,i,o].
Coefficient rows are broadcast across partitions with one-hot selector
matmuls on the PE; scaling is elementwise on DVE. The mixed bias
(coeff @ b_e) is folded in as an extra K-tile whose rhs is coeffT and
whose lhsT is the bias matrix.

ELU(x) = max(x, min(exp(x),1) - 1)   (exp monotonic => exp(min(x,0)) =
min(exp(x),1)); exp on ScalarE, the rest on Vector/GpSimd engines.
"""

import numpy as np
from contextlib import ExitStack

import concourse.bass as bass
import concourse.bacc as bacc
import concourse.tile as tile
import concourse.mybir as mybir
from concourse.bass_utils import run_bass_kernel_spmd

F16 = mybir.dt.float16
F32 = mybir.dt.float32
AOP = mybir.AluOpType
AF = mybir.ActivationFunctionType

B = 2048
NCORES = 8
BC = B // NCORES          # 256 batch rows per core
F = 267                   # frame size
L = 32                    # latent
H = 256                   # hidden
E = 8                     # experts
G = 64                    # gate hidden
IN0 = L + F               # 299
IN1 = L + H               # 288

LAST_RESULTS = None       # BassKernelResults of the most recent run
_CACHE = {}


def _eps42():
    """eps = jax.random.normal(key(42), (B, L)) exactly as the reference
    computes it, on the default jax backend (PRNG lowerings differ between
    backends, so we must mirror the reference's code path, not hardcode)."""
    if "eps" not in _CACHE:
        import jax
        import jax.numpy as jnp

        _CACHE["eps"] = np.asarray(
            jax.random.normal(jax.random.key(42), (B, L), jnp.float32)
        )
    return _CACHE["eps"]


def _pad_rows(a, rows):
    out = np.zeros((rows, a.shape[1]), a.dtype)
    out[: a.shape[0]] = a
    return out


def _bcast_ap(ap2d, reps):
    """[P, N] AP -> [P, reps, N] AP with a step-0 middle dim (operand bcast)."""
    return bass.AP(
        tensor=ap2d.tensor,
        offset=ap2d.offset,
        ap=[ap2d.ap[0], [0, reps], ap2d.ap[1]],
    )


def _build_program():
    nc = bacc.Bacc("TRN2")
    d = {}

    def din(name, shape, dt=F16):
        d[name] = nc.dram_tensor(name, shape, dt, kind="ExternalInput").ap()

    def dout(name, shape, dt=F32):
        d[name] = nc.dram_tensor(name, shape, dt, kind="ExternalOutput").ap()

    # per-core activations ([feat, batch] fp16; xo/co carry a trailing ones row)
    din("xo", [268, BC])
    din("co", [268, BC])
    din("epsT", [L, BC])
    # replicated weights (host-restacked, see kernel())
    din("sel", [8, 9, 128])
    din("ident", [128, 128], F32)
    din("fc1w", [6 * 128, 256])
    din("fc2w", [5 * 128, 256])
    din("mvw", [5 * 128, 64])
    din("g0w", [4 * 128, 64])
    din("g1w", [128, 64])
    din("g1wb", [1, 64])
    din("g2w", [128, 8])
    din("g2wb", [1, 8])
    din("w0st", [21 * 128, 256])
    din("w1st", [19 * 128, 256])
    din("w2st", [19 * 128, 267])
    dout("outT", [F, BC])
    dout("mvT", [2 * L, BC])

    with tile.TileContext(nc) as tc, ExitStack() as ctx:
        sb = ctx.enter_context(tc.tile_pool(name="sb", bufs=1))
        ps_big = ctx.enter_context(tc.tile_pool(name="ps_big", bufs=2, space="PSUM"))
        ps_sm = ctx.enter_context(tc.tile_pool(name="ps_sm", bufs=3, space="PSUM"))
        ps_bc = ctx.enter_context(tc.tile_pool(name="ps_bc", bufs=2, space="PSUM"))

        def sbt(tag, shape, dt=F16):
            return sb.tile(shape, dt, tag=tag, name=tag)

        # ---- input / weight DMAs -------------------------------------
        def dma_tiled(dst, src, nk, cols, eng=None):
            """DRAM [nk*128, cols] -> SBUF [128, nk, cols]."""
            eng = eng or nc.sync
            eng.dma_start(
                dst[:, 0:nk, :],
                src[0 : nk * 128, :].rearrange("(k p) m -> p k m", p=128),
            )

        # Spread DMAs over all five engines' queues so they pull in parallel,
        # first-needed tensors first on each queue.
        engs = [nc.sync, nc.scalar, nc.gpsimd]

        # DMA ordering is bandwidth-critical (~4.7MB/core at the shared HBM):
        # first-needed tensors first, split across both HWDGE queues; tiny
        # constants ride the SWDGE (gpsimd) queue.
        xo = sbt("xo", [128, 3, BC])
        nc.sync.dma_start(xo[:, 0:2, :], d["xo"][0:256, :].rearrange("(k p) n -> p k n", p=128))
        nc.gpsimd.dma_start(xo[0:12, 2, :], d["xo"][256:268, :])
        co = sbt("co", [128, 3, BC])
        nc.scalar.dma_start(co[:, 0:2, :], d["co"][0:256, :].rearrange("(k p) n -> p k n", p=128))
        nc.gpsimd.dma_start(co[0:12, 2, :], d["co"][256:268, :])
        fc1w = sbt("fc1w", [128, 6, 256])
        nc.sync.dma_start(fc1w[:, 0:3, :], d["fc1w"][0:384, :].rearrange("(k p) m -> p k m", p=128))
        nc.scalar.dma_start(fc1w[:, 3:6, :], d["fc1w"][384:768, :].rearrange("(k p) m -> p k m", p=128))
        fc2w = sbt("fc2w", [128, 5, 256])
        nc.sync.dma_start(fc2w[:, 0:3, :], d["fc2w"][0:384, :].rearrange("(k p) m -> p k m", p=128))
        nc.scalar.dma_start(fc2w[:, 3:5, :], d["fc2w"][384:640, :].rearrange("(k p) m -> p k m", p=128))
        eps = sbt("eps", [L, BC])
        nc.gpsimd.dma_start(eps[:], d["epsT"][:])
        sel = sbt("sel", [8, 9, 128])
        nc.gpsimd.dma_start(sel[:], d["sel"][:])
        ident = sbt("ident", [128, 128], F32)
        nc.gpsimd.dma_start(ident[:], d["ident"][:])

        mvw = sbt("mvw", [128, 5, 64])
        dma_tiled(mvw, d["mvw"], 5, 64, eng=nc.sync)
        g0w = sbt("g0w", [128, 4, 64])
        dma_tiled(g0w, d["g0w"], 4, 64, eng=nc.scalar)
        g1w = sbt("g1w", [128, 64])
        nc.gpsimd.dma_start(g1w[:], d["g1w"][:])
        g1wb = sbt("g1wb", [1, 64])
        nc.gpsimd.dma_start(g1wb[:], d["g1wb"][:])
        g2w = sbt("g2w", [128, 8])
        nc.gpsimd.dma_start(g2w[:], d["g2w"][:])
        g2wb = sbt("g2wb", [1, 8])
        nc.gpsimd.dma_start(g2wb[:], d["g2wb"][:])

        # decoder weights: chunked, alternating between the two HWDGE queues
        w0 = sbt("w0", [128, 21, 256])
        w1 = sbt("w1", [128, 19, 256])
        w2 = sbt("w2", [128, 19, 267])
        hw = [nc.sync, nc.scalar]
        qi = 0
        for (wt, src, nk) in ((w0, d["w0st"], 21), (w1, d["w1st"], 19), (w2, d["w2st"], 19)):
            for lo in range(0, nk, 4):
                hi = min(lo + 4, nk)
                hw[qi % 2].dma_start(
                    wt[:, lo:hi, :],
                    src[lo * 128 : hi * 128, :].rearrange("(k p) m -> p k m", p=128),
                )
                qi += 1

        # touch the exp table now so ACT_TABLE_LOAD overlaps the input DMAs
        scratch1 = sbt("scratch1", [1, 8])
        nc.scalar.activation(scratch1[:], sel[0:1, 8, 0:8], AF.Exp)

        # L0 c-tail scaled tiles: zero the 21-row pads once, up front
        s0t = sbt("s0t", [128, 2, BC])
        nc.vector.memset(s0t[:], 0.0)

        # ---- helpers -------------------------------------------------
        def elu(pp, width, out_ap, tagbase):
            """ELU(x) = relu(x) + (min(exp(x),1) - 1), x = psum pp [P, width]."""
            p = pp.shape[0]
            ex = sbt(tagbase + "_e", [p, width])
            nc.scalar.activation(ex[:], pp, AF.Exp)
            tt = sbt(tagbase + "_t", [p, width])
            nc.vector.tensor_scalar(tt[:], ex[:], 1.0, 1.0, AOP.min, AOP.subtract)
            nc.vector.scalar_tensor_tensor(out_ap, pp, 0.0, tt[:], AOP.max, AOP.add)

        # ---- encoder -------------------------------------------------
        with nc.named_scope("enc1"):
            ph1 = ps_big.tile([128, 512], F32, tag="pbig", name="pbig")
            enc1_rhs = [
                (xo[:, 0, :], slice(0, 128)),
                (xo[:, 1, :], slice(0, 128)),
                (xo[0:12, 2, :], slice(0, 12)),
                (co[:, 0, :], slice(0, 128)),
                (co[:, 1, :], slice(0, 128)),
                (co[0:12, 2, :], slice(0, 12)),
            ]
            for m in range(2):
                n = len(enc1_rhs)
                for i, (rh, krows) in enumerate(enc1_rhs):
                    nc.tensor.matmul(
                        ph1[:, m * BC : (m + 1) * BC],
                        fc1w[krows, i, m * 128 : (m + 1) * 128],
                        rh,
                        start=(i == 0),
                        stop=(i == n - 1),
                    )
            h1 = sbt("h1", [128, 2, BC])
            elu(ph1[:], 512, h1[:].rearrange("p k n -> p (k n)"), "eh1")

        with nc.named_scope("enc2"):
            ph2 = ps_big.tile([128, 512], F32, tag="pbig", name="pbig")
            enc2_rhs = [
                (xo[:, 0, :], slice(0, 128)),
                (xo[:, 1, :], slice(0, 128)),
                (xo[0:12, 2, :], slice(0, 12)),
                (h1[:, 0, :], slice(0, 128)),
                (h1[:, 1, :], slice(0, 128)),
            ]
            for m in range(2):
                n = len(enc2_rhs)
                for i, (rh, krows) in enumerate(enc2_rhs):
                    nc.tensor.matmul(
                        ph2[:, m * BC : (m + 1) * BC],
                        fc2w[krows, i, m * 128 : (m + 1) * 128],
                        rh,
                        start=(i == 0),
                        stop=(i == n - 1),
                    )
            h2 = sbt("h2", [128, 2, BC])
            elu(ph2[:], 512, h2[:].rearrange("p k n -> p (k n)"), "eh2")

        with nc.named_scope("muvar"):
            pmv = ps_sm.tile([64, BC], F32, tag="psm", name="psm")
            mv_rhs = [
                (xo[:, 0, :], slice(0, 128)),
                (xo[:, 1, :], slice(0, 128)),
                (xo[0:12, 2, :], slice(0, 12)),
                (h2[:, 0, :], slice(0, 128)),
                (h2[:, 1, :], slice(0, 128)),
            ]
            n = len(mv_rhs)
            for i, (rh, krows) in enumerate(mv_rhs):
                nc.tensor.matmul(pmv[:], mvw[krows, i, 0:64], rh, start=(i == 0), stop=(i == n - 1))
            mvf = sbt("mvf", [64, BC], F32)
            nc.scalar.copy(mvf[:], pmv[:])                 # fp32 mu/logvar out
            nc.sync.dma_start(d["mvT"][:], mvf[:])
            # z = mu + eps * exp(0.5*logvar)
            stdt = sbt("stdt", [L, BC])
            nc.scalar.activation(stdt[:], pmv[32:64, :], AF.Exp, scale=0.5)
            zt = sbt("zt", [L, BC])
            nc.vector.tensor_mul(zt[:], eps[:], stdt[:])
            z = sbt("z", [L, BC])
            nc.vector.tensor_add(z[:], zt[:], pmv[0:32, :])

        # ---- gate ----------------------------------------------------
        # Gate ELUs are folded into the next matmul:  W@elu(x) = W@relu(x)
        # + W@t with t = min(exp(x),1)-1, so the 64-row relu/t halves stack
        # into one 128-row K-tile (weights duplicated host-side) and the DVE
        # max-combine drops off the serial chain.
        with nc.named_scope("gate"):
            onesr = sbt("onesr", [1, BC])
            nc.vector.memset(onesr[:], 1.0)
            pg0 = ps_sm.tile([64, BC], F32, tag="psm", name="psm")
            # z-free tiles first so the PE can start before z is ready
            g0_rhs = [
                (co[:, 0, :], slice(0, 128)),
                (co[:, 1, :], slice(0, 128)),
                (co[0:12, 2, :], slice(0, 12)),
                (z[:], slice(0, 32)),
            ]
            n = len(g0_rhs)
            for i, (rh, krows) in enumerate(g0_rhs):
                nc.tensor.matmul(pg0[:], g0w[krows, i, 0:64], rh, start=(i == 0), stop=(i == n - 1))

            def gate_relu_t(pp, tagbase):
                ga = sbt(tagbase + "_a", [128, BC])
                ex = sbt(tagbase + "_e", [64, BC])
                nc.scalar.activation(ex[:], pp, AF.Exp)
                nc.scalar.activation(ga[0:64, :], pp, AF.Relu)
                nc.vector.tensor_scalar(ga[64:128, :], ex[:], 1.0, 1.0, AOP.min, AOP.subtract)
                return ga

            g1in = gate_relu_t(pg0[:], "eg0")
            pg1 = ps_sm.tile([64, BC], F32, tag="psm", name="psm")
            nc.tensor.matmul(pg1[:], g1w[:, 0:64], g1in[:], start=True, stop=False)
            nc.tensor.matmul(pg1[:], g1wb[0:1, 0:64], onesr[:], start=False, stop=True)

            g2in = gate_relu_t(pg1[:], "eg1")
            plg = ps_sm.tile([8, BC], F32, tag="psm", name="psm")
            nc.tensor.matmul(plg[:], g2w[:, 0:8], g2in[:], start=True, stop=False)
            nc.tensor.matmul(plg[:], g2wb[0:1, 0:8], onesr[:], start=False, stop=True)

            # softmax over the 8 experts (partition dim) without max-sub.
            # The 1/sum is computed batch-on-partitions so the DVE iterative
            # divide runs 2 elems/lane x 128 lanes instead of 256 on one lane.
            expE = sbt("expE", [8, BC])
            nc.scalar.activation(expE[:], plg[:], AF.Exp)
            pst = ps_sm.tile([128, 2], F32, tag="psm", name="psm")
            nc.tensor.matmul(pst[:, 0:1], expE[0:8, 0:128], sel[0:8, 8, 0:1], start=True, stop=True)
            nc.tensor.matmul(pst[:, 1:2], expE[0:8, 128:256], sel[0:8, 8, 0:1], start=True, stop=True)
            rst = sbt("rst", [128, 2], F32)
            with nc.allow_low_precision(reason="softmax denominators are well-conditioned"):
                nc.vector.reciprocal(rst[:], pst[:])
            precS = ps_sm.tile([1, BC], F32, tag="psm", name="psm")
            nc.tensor.matmul(precS[0:1, 0:128], rst[:, 0:1], ident[:], start=True, stop=True)
            nc.tensor.matmul(precS[0:1, 128:256], rst[:, 1:2], ident[:], start=True, stop=True)
            recS = sbt("recS", [1, BC])
            nc.scalar.copy(recS[:], precS[:])
            pr8 = ps_sm.tile([8, BC], F32, tag="psm", name="psm")
            nc.tensor.matmul(pr8[:], sel[0:1, 8, 0:8], recS[:], start=True, stop=True)
            coeffT = sbt("coeffT", [8, BC])
            nc.vector.tensor_mul(coeffT[:], expE[:], pr8[:])

            # broadcast each coeff row to 128 partitions: sel_e.T @ coeffT
            bcastC = sbt("bcastC", [128, 8, BC])
            for i in range(4):
                pbc = ps_bc.tile([128, 512], F32, tag="pbc", name="pbc")
                nc.tensor.matmul(pbc[:, 0:BC], sel[:, 2 * i, :], coeffT[:], start=True, stop=True)
                nc.tensor.matmul(pbc[:, BC:512], sel[:, 2 * i + 1, :], coeffT[:], start=True, stop=True)
                cpeng = nc.vector if i % 2 == 0 else nc.scalar
                if cpeng is nc.vector:
                    nc.vector.tensor_copy(bcastC[:, 2 * i : 2 * i + 2, :].rearrange("p k n -> p (k n)"), pbc[:])
                else:
                    nc.scalar.copy(bcastC[:, 2 * i : 2 * i + 2, :].rearrange("p k n -> p (k n)"), pbc[:])

        # ---- decoder -------------------------------------------------
        # z-part of the scaled stack (shared by all 3 layers)
        with nc.named_scope("zstack"):
            zs = sbt("zs", [128, 2, BC])
            for e in range(E):
                nc.vector.tensor_tensor(
                    zs[32 * (e % 4) : 32 * (e % 4) + 32, e // 4, :],
                    z[:],
                    bcastC[0:32, e, :],
                    AOP.mult,
                )

        def scaled_stack(name, src3, nsub):
            """Per-expert coefficient-scaled copies of src3 [128, nsub, BC]."""
            st = sbt(name, [128, E * nsub, BC])
            for e in range(E):
                nc.vector.tensor_tensor(
                    st[:, e * nsub : (e + 1) * nsub, :],
                    src3,
                    _bcast_ap(bcastC[:, e, :], nsub),
                    AOP.mult,
                )
            return st

        def decoder_layer(name, wt, stack_tiles, psum_w, mtiles, out_elu):
            """stack_tiles: list of (rhs_ap, krows) aligned with wt subtiles."""
            with nc.named_scope(name):
                pl = ps_big.tile([128, 512], F32, tag="pbig", name="pbig") if psum_w == 512 else None
                outs = []
                for mi, (mlo, mhi) in enumerate(mtiles):
                    if mhi - mlo == 128 and pl is not None:
                        pap = pl[:, mi * BC : (mi + 1) * BC]
                    else:
                        ptail = ps_sm.tile([mhi - mlo, BC], F32, tag="psm", name="psm")
                        pap = ptail[:]
                        outs.append(ptail)
                    n = len(stack_tiles)
                    for i, (rh, krows) in enumerate(stack_tiles):
                        nc.tensor.matmul(
                            pap,
                            wt[krows, i, mlo:mhi],
                            rh,
                            start=(i == 0),
                            stop=(i == n - 1),
                        )
                if out_elu is not None:
                    elu(pl[:], 512, out_elu[:].rearrange("p k n -> p (k n)"), "e" + name)
                return pl, outs

        with nc.named_scope("l0scale"):
            s0 = scaled_stack("s0", co[:, 0:2, :], 2)
            # c tail rows (11 per expert, padded to 32-partition blocks: compute
            # engines require 32-aligned partition bases)
            for e in range(E):
                nc.vector.tensor_tensor(
                    s0t[32 * (e % 4) : 32 * (e % 4) + 11, e // 4, :],
                    co[0:11, 2, :],
                    bcastC[0:11, e, :],
                    AOP.mult,
                )

        l0_tiles = [(s0[:, j, :], slice(0, 128)) for j in range(16)]
        l0_tiles += [(s0t[:, 0, :], slice(0, 128)), (s0t[:, 1, :], slice(0, 128))]
        l0_tiles += [(zs[:, 0, :], slice(0, 128)), (zs[:, 1, :], slice(0, 128)), (coeffT[:], slice(0, 8))]
        l0o = sbt("l0o", [128, 2, BC])
        decoder_layer("l0", w0, l0_tiles, 512, [(0, 128), (128, 256)], l0o)

        with nc.named_scope("l1scale"):
            s1 = scaled_stack("s1", l0o[:, 0:2, :], 2)
        l1_tiles = [(s1[:, j, :], slice(0, 128)) for j in range(16)]
        l1_tiles += [(zs[:, 0, :], slice(0, 128)), (zs[:, 1, :], slice(0, 128)), (coeffT[:], slice(0, 8))]
        l1o = sbt("l1o", [128, 2, BC])
        decoder_layer("l1", w1, l1_tiles, 512, [(0, 128), (128, 256)], l1o)

        with nc.named_scope("l2scale"):
            s2 = scaled_stack("s2", l1o[:, 0:2, :], 2)
        l2_tiles = [(s2[:, j, :], slice(0, 128)) for j in range(16)]
        l2_tiles += [(zs[:, 0, :], slice(0, 128)), (zs[:, 1, :], slice(0, 128)), (coeffT[:], slice(0, 8))]
        pl2, tails = decoder_layer(
            "l2", w2, l2_tiles, 512, [(0, 128), (128, 256), (256, 267)], None
        )

        with nc.named_scope("out"):
            out0 = sbt("out0", [128, 512], F32)
            nc.scalar.copy(out0[:, 0:BC], pl2[:, 0:BC])
            nc.sync.dma_start(d["outT"][0:128, :], out0[:, 0:BC])
            nc.vector.tensor_copy(out0[:, BC:512], pl2[:, BC:512])
            nc.scalar.dma_start(d["outT"][128:256, :], out0[:, BC:512])
            out2 = sbt("out2", [11, BC], F32)
            nc.scalar.copy(out2[:], tails[0][:])
            nc.sync.dma_start(d["outT"][256:267, :], out2[:])

    nc.compile()
    return nc


def _host_weights(i):
    """Restack/transpose/cast all weights for the device layout."""
    f16 = np.float16

    def t(a):
        return np.asarray(a, np.float32).T  # [in, out]

    W1t = t(i["fc1_w"])  # [534, 256]
    fc1 = np.concatenate(
        [
            W1t[0:128],
            W1t[128:256],
            _pad_rows(np.concatenate([W1t[256:267], i["fc1_b"][None, :]], 0), 128),
            W1t[267:395],
            W1t[395:523],
            _pad_rows(W1t[523:534], 128),
        ],
        0,
    )
    W2t = t(i["fc2_w"])  # [523, 256]
    fc2 = np.concatenate(
        [
            W2t[0:128],
            W2t[128:256],
            _pad_rows(np.concatenate([W2t[256:267], i["fc2_b"][None, :]], 0), 128),
            W2t[267:395],
            W2t[395:523],
        ],
        0,
    )
    Wmv = np.concatenate([t(i["mu_w"]), t(i["lv_w"])], 1)  # [523, 64]
    bmv = np.concatenate([i["mu_b"], i["lv_b"]])[None, :]
    mv = np.concatenate(
        [
            Wmv[0:128],
            Wmv[128:256],
            _pad_rows(np.concatenate([Wmv[256:267], bmv], 0), 128),
            Wmv[267:395],
            Wmv[395:523],
        ],
        0,
    )
    G0 = t(i["g0_w"])  # [299, 64]
    g0 = np.concatenate(
        [
            G0[32:160],
            G0[160:288],
            _pad_rows(np.concatenate([G0[288:299], i["g0_b"][None, :]], 0), 128),
            _pad_rows(G0[0:32], 128),
        ],
        0,
    )
    g1 = np.concatenate([t(i["g1_w"]), t(i["g1_w"])], 0)  # relu-half + t-half
    g2 = np.concatenate([t(i["g2_w"]), t(i["g2_w"])], 0)

    def dec_stack(w, b):
        w = np.asarray(w, np.float32)  # [E, in, out]
        parts = []
        for e in range(E):
            parts.append(w[e, 32:160])
            parts.append(w[e, 160:288])
        if w.shape[1] == IN0:  # layer 0: c tail rows, 32-row block per expert
            for g in range(2):
                parts.append(
                    np.concatenate(
                        [_pad_rows(w[e, 288:299], 32) for e in range(4 * g, 4 * g + 4)], 0
                    )
                )
        parts.append(np.concatenate([w[e, 0:32] for e in range(4)], 0))
        parts.append(np.concatenate([w[e, 0:32] for e in range(4, 8)], 0))
        parts.append(_pad_rows(np.asarray(b, np.float32), 128))
        return np.concatenate(parts, 0)

    w0st = dec_stack(i["w0"], i["b0"])
    w1st = dec_stack(i["w1"], i["b1"])
    w2st = dec_stack(i["w2"], i["b2"])

    sel = np.zeros((8, 9, 128), np.float32)
    for e in range(E):
        sel[e, e, :] = 1.0
    sel[:, 8, :] = 1.0

    return {
        "ident": np.eye(128, dtype=np.float32),
        "sel": sel.astype(f16),
        "fc1w": fc1.astype(f16),
        "fc2w": fc2.astype(f16),
        "mvw": mv.astype(f16),
        "g0w": g0.astype(f16),
        "g1w": g1.astype(f16),
        "g1wb": np.asarray(i["g1_b"], np.float32)[None, :].astype(f16),
        "g2w": g2.astype(f16),
        "g2wb": np.asarray(i["g2_b"], np.float32)[None, :].astype(f16),
        "w0st": w0st.astype(f16),
        "w1st": w1st.astype(f16),
        "w2st": w2st.astype(f16),
    }


def kernel(**inputs):
    global LAST_RESULTS
    if "nc" not in _CACHE:
        _CACHE["nc"] = _build_program()
    nc = _CACHE["nc"]

    i = {k: np.asarray(v) for k, v in inputs.items()}
    eps = _eps42()
    wmap = _host_weights(i)

    ones = np.ones((1, B), np.float32)
    xo_full = np.concatenate([np.asarray(i["x"], np.float32).T, ones], 0).astype(np.float16)
    co_full = np.concatenate([np.asarray(i["c"], np.float32).T, ones], 0).astype(np.float16)
    epsT = eps.T.astype(np.float16)

    in_maps = []
    for ci in range(NCORES):
        s = slice(ci * BC, (ci + 1) * BC)
        m = dict(wmap)
        m["xo"] = np.ascontiguousarray(xo_full[:, s])
        m["co"] = np.ascontiguousarray(co_full[:, s])
        m["epsT"] = np.ascontiguousarray(epsT[:, s])
        in_maps.append(m)

    res = run_bass_kernel_spmd(nc, in_maps, core_ids=list(range(NCORES)))
    LAST_RESULTS = res

    out = np.empty((B, F), np.float32)
    mu = np.empty((B, L), np.float32)
    lv = np.empty((B, L), np.float32)
    for ci in range(NCORES):
        s = slice(ci * BC, (ci + 1) * BC)
        r = res.results[ci]
        out[s] = r["outT"].T
        mu[s] = r["mvT"][0:L].T
        lv[s] = r["mvT"][L : 2 * L].T
    return out, mu, lv


# revision 36
# speedup vs baseline: 1.1074x; 1.0756x over previous
"""Trainium2 Bass kernel for CustomPoseMixtureVAE (moe_routing).

Strategy: data-parallel over batch across 8 NeuronCores (256 rows/core),
all weights replicated, no collectives. Activations kept feature-major
[feat, batch] on-chip so every linear is a single PSUM-accumulated GEMM
chain with the (host-pretransposed) weight as the stationary operand.

The expert mixture  out = einsum('be,bi,eio->bo', coeff, inp, W)  is
computed as ONE GEMM over K = E*in using per-expert coefficient-scaled
inputs stacked along K:  out[b,o] = sum_{e,i} (coeff[b,e]*inp[b,i]) W[e,i,o].
Coefficient rows are broadcast across partitions with one-hot selector
matmuls on the PE; scaling is elementwise on DVE. The mixed bias
(coeff @ b_e) is folded in as an extra K-tile whose rhs is coeffT and
whose lhsT is the bias matrix.

ELU(x) = max(x, min(exp(x),1) - 1)   (exp monotonic => exp(min(x,0)) =
min(exp(x),1)); exp on ScalarE, the rest on Vector/GpSimd engines.
"""

import numpy as np
from contextlib import ExitStack

import concourse.bass as bass
import concourse.bacc as bacc
import concourse.tile as tile
import concourse.mybir as mybir
from concourse.bass_utils import run_bass_kernel_spmd

F16 = mybir.dt.float16
F32 = mybir.dt.float32
AOP = mybir.AluOpType
AF = mybir.ActivationFunctionType

B = 2048
NCORES = 8
BC = B // NCORES          # 256 batch rows per core
F = 267                   # frame size
L = 32                    # latent
H = 256                   # hidden
E = 8                     # experts
G = 64                    # gate hidden
IN0 = L + F               # 299
IN1 = L + H               # 288

LAST_RESULTS = None       # BassKernelResults of the most recent run
_CACHE = {}


def _eps42():
    """eps = jax.random.normal(key(42), (B, L)) exactly as the reference
    computes it, on the default jax backend (PRNG lowerings differ between
    backends, so we must mirror the reference's code path, not hardcode)."""
    if "eps" not in _CACHE:
        import jax
        import jax.numpy as jnp

        _CACHE["eps"] = np.asarray(
            jax.random.normal(jax.random.key(42), (B, L), jnp.float32)
        )
    return _CACHE["eps"]


def _pad_rows(a, rows):
    out = np.zeros((rows, a.shape[1]), a.dtype)
    out[: a.shape[0]] = a
    return out


def _bcast_ap(ap2d, reps):
    """[P, N] AP -> [P, reps, N] AP with a step-0 middle dim (operand bcast)."""
    return bass.AP(
        tensor=ap2d.tensor,
        offset=ap2d.offset,
        ap=[ap2d.ap[0], [0, reps], ap2d.ap[1]],
    )


def _build_program():
    nc = bacc.Bacc("TRN2")
    d = {}

    def din(name, shape, dt=F16):
        d[name] = nc.dram_tensor(name, shape, dt, kind="ExternalInput").ap()

    def dout(name, shape, dt=F32):
        d[name] = nc.dram_tensor(name, shape, dt, kind="ExternalOutput").ap()

    # per-core activations ([feat, batch] fp16; xo/co carry a trailing ones row)
    din("xo", [268, BC])
    din("co", [268, BC])
    din("epsT", [L, BC])
    # replicated weights (host-restacked, see kernel())
    din("sel", [8, 9, 128])
    din("ident", [128, 128], F32)
    din("fc1w", [6 * 128, 256])
    din("fc2w", [5 * 128, 256])
    din("mvw", [5 * 128, 64])
    din("g0w", [4 * 128, 64])
    din("g1w", [128, 64])
    din("g1wb", [1, 64])
    din("g2w", [128, 8])
    din("g2wb", [1, 8])
    din("w0st", [21 * 128, 256])
    din("w1st", [19 * 128, 256])
    din("w2st", [19 * 128, 267])
    dout("outT", [F, BC])
    dout("mvT", [2 * L, BC])

    with tile.TileContext(nc) as tc, ExitStack() as ctx:
        sb = ctx.enter_context(tc.tile_pool(name="sb", bufs=1))
        ps_big = ctx.enter_context(tc.tile_pool(name="ps_big", bufs=2, space="PSUM"))
        ps_sm = ctx.enter_context(tc.tile_pool(name="ps_sm", bufs=3, space="PSUM"))
        ps_bc = ctx.enter_context(tc.tile_pool(name="ps_bc", bufs=2, space="PSUM"))

        def sbt(tag, shape, dt=F16):
            return sb.tile(shape, dt, tag=tag, name=tag)

        # ---- input / weight DMAs -------------------------------------
        def dma_tiled(dst, src, nk, cols, eng=None):
            """DRAM [nk*128, cols] -> SBUF [128, nk, cols]."""
            eng = eng or nc.sync
            eng.dma_start(
                dst[:, 0:nk, :],
                src[0 : nk * 128, :].rearrange("(k p) m -> p k m", p=128),
            )

        # Spread DMAs over all five engines' queues so they pull in parallel,
        # first-needed tensors first on each queue.
        engs = [nc.sync, nc.scalar, nc.gpsimd]

        # DMA ordering is bandwidth-critical (~4.7MB/core at the shared HBM):
        # first-needed tensors first, split across both HWDGE queues; tiny
        # constants ride the SWDGE (gpsimd) queue.
        xo = sbt("xo", [128, 3, BC])
        nc.sync.dma_start(xo[:, 0:2, :], d["xo"][0:256, :].rearrange("(k p) n -> p k n", p=128))
        nc.gpsimd.dma_start(xo[0:12, 2, :], d["xo"][256:268, :])
        co = sbt("co", [128, 3, BC])
        nc.scalar.dma_start(co[:, 0:2, :], d["co"][0:256, :].rearrange("(k p) n -> p k n", p=128))
        nc.gpsimd.dma_start(co[0:12, 2, :], d["co"][256:268, :])
        fc1w = sbt("fc1w", [128, 6, 256])
        nc.sync.dma_start(fc1w[:, 0:3, :], d["fc1w"][0:384, :].rearrange("(k p) m -> p k m", p=128))
        nc.scalar.dma_start(fc1w[:, 3:6, :], d["fc1w"][384:768, :].rearrange("(k p) m -> p k m", p=128))
        fc2w = sbt("fc2w", [128, 5, 256])
        nc.sync.dma_start(fc2w[:, 0:3, :], d["fc2w"][0:384, :].rearrange("(k p) m -> p k m", p=128))
        nc.scalar.dma_start(fc2w[:, 3:5, :], d["fc2w"][384:640, :].rearrange("(k p) m -> p k m", p=128))
        eps = sbt("eps", [L, BC])
        nc.gpsimd.dma_start(eps[:], d["epsT"][:])
        sel = sbt("sel", [8, 9, 128])
        nc.gpsimd.dma_start(sel[:], d["sel"][:])
        ident = sbt("ident", [128, 128], F32)
        nc.gpsimd.dma_start(ident[:], d["ident"][:])

        # touch the exp table now so ACT_TABLE_LOAD overlaps the input DMAs.
        # NOTE: DMA issue occupies the issuing engine's FIFO for ~650ns per
        # dma_start — the Scalar engine must stay free for ACT compute, so
        # everything below goes on sync (idle) or the gpsimd SWDGE.
        scratch1 = sbt("scratch1", [1, 8])
        nc.scalar.activation(scratch1[:], sel[0:1, 8, 0:8], AF.Exp)

        mvw = sbt("mvw", [128, 5, 64])
        dma_tiled(mvw, d["mvw"], 5, 64, eng=nc.gpsimd)
        g0w = sbt("g0w", [128, 4, 64])
        dma_tiled(g0w, d["g0w"], 4, 64, eng=nc.gpsimd)
        g1w = sbt("g1w", [128, 64])
        nc.gpsimd.dma_start(g1w[:], d["g1w"][:])
        g1wb = sbt("g1wb", [1, 64])
        nc.gpsimd.dma_start(g1wb[:], d["g1wb"][:])
        g2w = sbt("g2w", [128, 8])
        nc.gpsimd.dma_start(g2w[:], d["g2w"][:])
        g2wb = sbt("g2wb", [1, 8])
        nc.gpsimd.dma_start(g2wb[:], d["g2wb"][:])

        # decoder weights: few large transfers, all issued by the idle sync
        # engine; the dynamic HWDGE pool streams them in the background
        w0 = sbt("w0", [128, 21, 256])
        w1 = sbt("w1", [128, 19, 256])
        w2 = sbt("w2", [128, 19, 267])
        for (wt, src, nk) in ((w0, d["w0st"], 21), (w1, d["w1st"], 19), (w2, d["w2st"], 19)):
            mid = (nk + 1) // 2
            for lo, hi in ((0, mid), (mid, nk)):
                nc.sync.dma_start(
                    wt[:, lo:hi, :],
                    src[lo * 128 : hi * 128, :].rearrange("(k p) m -> p k m", p=128),
                )

        # L0 c-tail scaled tiles: zero the 21-row pads once, up front
        s0t = sbt("s0t", [128, 2, BC])
        nc.vector.memset(s0t[:], 0.0)

        # ---- helpers -------------------------------------------------
        def elu(pp, width, out_ap, tagbase):
            """ELU(x) = relu(x) + (min(exp(x),1) - 1), x = psum pp [P, width]."""
            p = pp.shape[0]
            ex = sbt(tagbase + "_e", [p, width])
            nc.scalar.activation(ex[:], pp, AF.Exp)
            tt = sbt(tagbase + "_t", [p, width])
            nc.vector.tensor_scalar(tt[:], ex[:], 1.0, 1.0, AOP.min, AOP.subtract)
            nc.vector.scalar_tensor_tensor(out_ap, pp, 0.0, tt[:], AOP.max, AOP.add)

        # ---- encoder -------------------------------------------------
        with nc.named_scope("enc1"):
            ph1 = ps_big.tile([128, 512], F32, tag="pbig", name="pbig")
            enc1_rhs = [
                (xo[:, 0, :], slice(0, 128)),
                (xo[:, 1, :], slice(0, 128)),
                (xo[0:12, 2, :], slice(0, 12)),
                (co[:, 0, :], slice(0, 128)),
                (co[:, 1, :], slice(0, 128)),
                (co[0:12, 2, :], slice(0, 12)),
            ]
            for m in range(2):
                n = len(enc1_rhs)
                for i, (rh, krows) in enumerate(enc1_rhs):
                    nc.tensor.matmul(
                        ph1[:, m * BC : (m + 1) * BC],
                        fc1w[krows, i, m * 128 : (m + 1) * 128],
                        rh,
                        start=(i == 0),
                        stop=(i == n - 1),
                    )
            h1 = sbt("h1", [128, 2, BC])
            elu(ph1[:], 512, h1[:].rearrange("p k n -> p (k n)"), "eh1")

        with nc.named_scope("enc2"):
            ph2 = ps_big.tile([128, 512], F32, tag="pbig", name="pbig")
            enc2_rhs = [
                (xo[:, 0, :], slice(0, 128)),
                (xo[:, 1, :], slice(0, 128)),
                (xo[0:12, 2, :], slice(0, 12)),
                (h1[:, 0, :], slice(0, 128)),
                (h1[:, 1, :], slice(0, 128)),
            ]
            for m in range(2):
                n = len(enc2_rhs)
                for i, (rh, krows) in enumerate(enc2_rhs):
                    nc.tensor.matmul(
                        ph2[:, m * BC : (m + 1) * BC],
                        fc2w[krows, i, m * 128 : (m + 1) * 128],
                        rh,
                        start=(i == 0),
                        stop=(i == n - 1),
                    )
            h2 = sbt("h2", [128, 2, BC])
            elu(ph2[:], 512, h2[:].rearrange("p k n -> p (k n)"), "eh2")

        with nc.named_scope("muvar"):
            pmv = ps_sm.tile([64, BC], F32, tag="psm", name="psm")
            mv_rhs = [
                (xo[:, 0, :], slice(0, 128)),
                (xo[:, 1, :], slice(0, 128)),
                (xo[0:12, 2, :], slice(0, 12)),
                (h2[:, 0, :], slice(0, 128)),
                (h2[:, 1, :], slice(0, 128)),
            ]
            n = len(mv_rhs)
            for i, (rh, krows) in enumerate(mv_rhs):
                nc.tensor.matmul(pmv[:], mvw[krows, i, 0:64], rh, start=(i == 0), stop=(i == n - 1))
            mvf = sbt("mvf", [64, BC], F32)
            nc.scalar.copy(mvf[:], pmv[:])                 # fp32 mu/logvar out
            nc.sync.dma_start(d["mvT"][:], mvf[:])
            # z = mu + eps * exp(0.5*logvar)
            stdt = sbt("stdt", [L, BC])
            nc.scalar.activation(stdt[:], pmv[32:64, :], AF.Exp, scale=0.5)
            zt = sbt("zt", [L, BC])
            nc.vector.tensor_mul(zt[:], eps[:], stdt[:])
            z = sbt("z", [L, BC])
            nc.vector.tensor_add(z[:], zt[:], pmv[0:32, :])

        # ---- gate ----------------------------------------------------
        # Gate ELUs are folded into the next matmul:  W@elu(x) = W@relu(x)
        # + W@t with t = min(exp(x),1)-1, so the 64-row relu/t halves stack
        # into one 128-row K-tile (weights duplicated host-side) and the DVE
        # max-combine drops off the serial chain.
        with nc.named_scope("gate"):
            onesr = sbt("onesr", [1, BC])
            nc.vector.memset(onesr[:], 1.0)
            pg0 = ps_sm.tile([64, BC], F32, tag="psm", name="psm")
            # z-free tiles first so the PE can start before z is ready
            g0_rhs = [
                (co[:, 0, :], slice(0, 128)),
                (co[:, 1, :], slice(0, 128)),
                (co[0:12, 2, :], slice(0, 12)),
                (z[:], slice(0, 32)),
            ]
            n = len(g0_rhs)
            for i, (rh, krows) in enumerate(g0_rhs):
                nc.tensor.matmul(pg0[:], g0w[krows, i, 0:64], rh, start=(i == 0), stop=(i == n - 1))

            def gate_relu_t(pp, tagbase):
                ga = sbt(tagbase + "_a", [128, BC])
                ex = sbt(tagbase + "_e", [64, BC])
                nc.scalar.activation(ex[:], pp, AF.Exp)
                nc.scalar.activation(ga[0:64, :], pp, AF.Relu)
                nc.vector.tensor_scalar(ga[64:128, :], ex[:], 1.0, 1.0, AOP.min, AOP.subtract)
                return ga

            g1in = gate_relu_t(pg0[:], "eg0")
            pg1 = ps_sm.tile([64, BC], F32, tag="psm", name="psm")
            nc.tensor.matmul(pg1[:], g1w[:, 0:64], g1in[:], start=True, stop=False)
            nc.tensor.matmul(pg1[:], g1wb[0:1, 0:64], onesr[:], start=False, stop=True)

            g2in = gate_relu_t(pg1[:], "eg1")
            plg = ps_sm.tile([8, BC], F32, tag="psm", name="psm")
            nc.tensor.matmul(plg[:], g2w[:, 0:8], g2in[:], start=True, stop=False)
            nc.tensor.matmul(plg[:], g2wb[0:1, 0:8], onesr[:], start=False, stop=True)

            # softmax over the 8 experts (partition dim) without max-sub.
            # The 1/sum is computed batch-on-partitions so the DVE iterative
            # divide runs 2 elems/lane x 128 lanes instead of 256 on one lane.
            expE = sbt("expE", [8, BC])
            nc.scalar.activation(expE[:], plg[:], AF.Exp)
            pst = ps_sm.tile([128, 2], F32, tag="psm", name="psm")
            nc.tensor.matmul(pst[:, 0:1], expE[0:8, 0:128], sel[0:8, 8, 0:1], start=True, stop=True)
            nc.tensor.matmul(pst[:, 1:2], expE[0:8, 128:256], sel[0:8, 8, 0:1], start=True, stop=True)
            rst = sbt("rst", [128, 2], F32)
            with nc.allow_low_precision(reason="softmax denominators are well-conditioned"):
                nc.vector.reciprocal(rst[:], pst[:])
            precS = ps_sm.tile([1, BC], F32, tag="psm", name="psm")
            nc.tensor.matmul(precS[0:1, 0:128], rst[:, 0:1], ident[:], start=True, stop=True)
            nc.tensor.matmul(precS[0:1, 128:256], rst[:, 1:2], ident[:], start=True, stop=True)
            recS = sbt("recS", [1, BC])
            nc.scalar.copy(recS[:], precS[:])
            pr8 = ps_sm.tile([8, BC], F32, tag="psm", name="psm")
            nc.tensor.matmul(pr8[:], sel[0:1, 8, 0:8], recS[:], start=True, stop=True)
            coeffT = sbt("coeffT", [8, BC])
            nc.vector.tensor_mul(coeffT[:], expE[:], pr8[:])

            # broadcast each coeff row to 128 partitions: sel_e.T @ coeffT
            bcastC = sbt("bcastC", [128, 8, BC])
            for i in range(4):
                pbc = ps_bc.tile([128, 512], F32, tag="pbc", name="pbc")
                nc.tensor.matmul(pbc[:, 0:BC], sel[:, 2 * i, :], coeffT[:], start=True, stop=True)
                nc.tensor.matmul(pbc[:, BC:512], sel[:, 2 * i + 1, :], coeffT[:], start=True, stop=True)
                cpeng = nc.vector if i % 2 == 0 else nc.scalar
                if cpeng is nc.vector:
                    nc.vector.tensor_copy(bcastC[:, 2 * i : 2 * i + 2, :].rearrange("p k n -> p (k n)"), pbc[:])
                else:
                    nc.scalar.copy(bcastC[:, 2 * i : 2 * i + 2, :].rearrange("p k n -> p (k n)"), pbc[:])

        # ---- decoder -------------------------------------------------
        # z-part of the scaled stack (shared by all 3 layers)
        with nc.named_scope("zstack"):
            zs = sbt("zs", [128, 2, BC])
            for e in range(E):
                nc.vector.tensor_tensor(
                    zs[32 * (e % 4) : 32 * (e % 4) + 32, e // 4, :],
                    z[:],
                    bcastC[0:32, e, :],
                    AOP.mult,
                )

        def scaled_stack(name, src3, nsub):
            """Per-expert coefficient-scaled copies of src3 [128, nsub, BC]."""
            st = sbt(name, [128, E * nsub, BC])
            for e in range(E):
                nc.vector.tensor_tensor(
                    st[:, e * nsub : (e + 1) * nsub, :],
                    src3,
                    _bcast_ap(bcastC[:, e, :], nsub),
                    AOP.mult,
                )
            return st

        def decoder_layer(name, wt, stack_tiles, psum_w, mtiles, out_elu):
            """stack_tiles: list of (rhs_ap, krows) aligned with wt subtiles."""
            with nc.named_scope(name):
                pl = ps_big.tile([128, 512], F32, tag="pbig", name="pbig") if psum_w == 512 else None
                outs = []
                for mi, (mlo, mhi) in enumerate(mtiles):
                    if mhi - mlo == 128 and pl is not None:
                        pap = pl[:, mi * BC : (mi + 1) * BC]
                    else:
                        ptail = ps_sm.tile([mhi - mlo, BC], F32, tag="psm", name="psm")
                        pap = ptail[:]
                        outs.append(ptail)
                    n = len(stack_tiles)
                    for i, (rh, krows) in enumerate(stack_tiles):
                        nc.tensor.matmul(
                            pap,
                            wt[krows, i, mlo:mhi],
                            rh,
                            start=(i == 0),
                            stop=(i == n - 1),
                        )
                if out_elu is not None:
                    elu(pl[:], 512, out_elu[:].rearrange("p k n -> p (k n)"), "e" + name)
                return pl, outs

        with nc.named_scope("l0scale"):
            s0 = scaled_stack("s0", co[:, 0:2, :], 2)
            # c tail rows (11 per expert, padded to 32-partition blocks: compute
            # engines require 32-aligned partition bases)
            for e in range(E):
                nc.vector.tensor_tensor(
                    s0t[32 * (e % 4) : 32 * (e % 4) + 11, e // 4, :],
                    co[0:11, 2, :],
                    bcastC[0:11, e, :],
                    AOP.mult,
                )

        l0_tiles = [(s0[:, j, :], slice(0, 128)) for j in range(16)]
        l0_tiles += [(s0t[:, 0, :], slice(0, 128)), (s0t[:, 1, :], slice(0, 128))]
        l0_tiles += [(zs[:, 0, :], slice(0, 128)), (zs[:, 1, :], slice(0, 128)), (coeffT[:], slice(0, 8))]
        l0o = sbt("l0o", [128, 2, BC])
        decoder_layer("l0", w0, l0_tiles, 512, [(0, 128), (128, 256)], l0o)

        with nc.named_scope("l1scale"):
            s1 = scaled_stack("s1", l0o[:, 0:2, :], 2)
        l1_tiles = [(s1[:, j, :], slice(0, 128)) for j in range(16)]
        l1_tiles += [(zs[:, 0, :], slice(0, 128)), (zs[:, 1, :], slice(0, 128)), (coeffT[:], slice(0, 8))]
        l1o = sbt("l1o", [128, 2, BC])
        decoder_layer("l1", w1, l1_tiles, 512, [(0, 128), (128, 256)], l1o)

        with nc.named_scope("l2scale"):
            s2 = scaled_stack("s2", l1o[:, 0:2, :], 2)
        l2_tiles = [(s2[:, j, :], slice(0, 128)) for j in range(16)]
        l2_tiles += [(zs[:, 0, :], slice(0, 128)), (zs[:, 1, :], slice(0, 128)), (coeffT[:], slice(0, 8))]
        pl2, tails = decoder_layer(
            "l2", w2, l2_tiles, 512, [(0, 128), (128, 256), (256, 267)], None
        )

        with nc.named_scope("out"):
            out0 = sbt("out0", [128, 512], F32)
            nc.scalar.copy(out0[:, 0:BC], pl2[:, 0:BC])
            nc.sync.dma_start(d["outT"][0:128, :], out0[:, 0:BC])
            nc.vector.tensor_copy(out0[:, BC:512], pl2[:, BC:512])
            nc.scalar.dma_start(d["outT"][128:256, :], out0[:, BC:512])
            out2 = sbt("out2", [11, BC], F32)
            nc.scalar.copy(out2[:], tails[0][:])
            nc.sync.dma_start(d["outT"][256:267, :], out2[:])

    nc.compile()
    return nc


def _host_weights(i):
    """Restack/transpose/cast all weights for the device layout."""
    f16 = np.float16

    def t(a):
        return np.asarray(a, np.float32).T  # [in, out]

    W1t = t(i["fc1_w"])  # [534, 256]
    fc1 = np.concatenate(
        [
            W1t[0:128],
            W1t[128:256],
            _pad_rows(np.concatenate([W1t[256:267], i["fc1_b"][None, :]], 0), 128),
            W1t[267:395],
            W1t[395:523],
            _pad_rows(W1t[523:534], 128),
        ],
        0,
    )
    W2t = t(i["fc2_w"])  # [523, 256]
    fc2 = np.concatenate(
        [
            W2t[0:128],
            W2t[128:256],
            _pad_rows(np.concatenate([W2t[256:267], i["fc2_b"][None, :]], 0), 128),
            W2t[267:395],
            W2t[395:523],
        ],
        0,
    )
    Wmv = np.concatenate([t(i["mu_w"]), t(i["lv_w"])], 1)  # [523, 64]
    bmv = np.concatenate([i["mu_b"], i["lv_b"]])[None, :]
    mv = np.concatenate(
        [
            Wmv[0:128],
            Wmv[128:256],
            _pad_rows(np.concatenate([Wmv[256:267], bmv], 0), 128),
            Wmv[267:395],
            Wmv[395:523],
        ],
        0,
    )
    G0 = t(i["g0_w"])  # [299, 64]
    g0 = np.concatenate(
        [
            G0[32:160],
            G0[160:288],
            _pad_rows(np.concatenate([G0[288:299], i["g0_b"][None, :]], 0), 128),
            _pad_rows(G0[0:32], 128),
        ],
        0,
    )
    g1 = np.concatenate([t(i["g1_w"]), t(i["g1_w"])], 0)  # relu-half + t-half
    g2 = np.concatenate([t(i["g2_w"]), t(i["g2_w"])], 0)

    def dec_stack(w, b):
        w = np.asarray(w, np.float32)  # [E, in, out]
        parts = []
        for e in range(E):
            parts.append(w[e, 32:160])
            parts.append(w[e, 160:288])
        if w.shape[1] == IN0:  # layer 0: c tail rows, 32-row block per expert
            for g in range(2):
                parts.append(
                    np.concatenate(
                        [_pad_rows(w[e, 288:299], 32) for e in range(4 * g, 4 * g + 4)], 0
                    )
                )
        parts.append(np.concatenate([w[e, 0:32] for e in range(4)], 0))
        parts.append(np.concatenate([w[e, 0:32] for e in range(4, 8)], 0))
        parts.append(_pad_rows(np.asarray(b, np.float32), 128))
        return np.concatenate(parts, 0)

    w0st = dec_stack(i["w0"], i["b0"])
    w1st = dec_stack(i["w1"], i["b1"])
    w2st = dec_stack(i["w2"], i["b2"])

    sel = np.zeros((8, 9, 128), np.float32)
    for e in range(E):
        sel[e, e, :] = 1.0
    sel[:, 8, :] = 1.0

    return {
        "ident": np.eye(128, dtype=np.float32),
        "sel": sel.astype(f16),
        "fc1w": fc1.astype(f16),
        "fc2w": fc2.astype(f16),
        "mvw": mv.astype(f16),
        "g0w": g0.astype(f16),
        "g1w": g1.astype(f16),
        "g1wb": np.asarray(i["g1_b"], np.float32)[None, :].astype(f16),
        "g2w": g2.astype(f16),
        "g2wb": np.asarray(i["g2_b"], np.float32)[None, :].astype(f16),
        "w0st": w0st.astype(f16),
        "w1st": w1st.astype(f16),
        "w2st": w2st.astype(f16),
    }


def kernel(**inputs):
    global LAST_RESULTS
    if "nc" not in _CACHE:
        _CACHE["nc"] = _build_program()
    nc = _CACHE["nc"]

    i = {k: np.asarray(v) for k, v in inputs.items()}
    eps = _eps42()
    wmap = _host_weights(i)

    ones = np.ones((1, B), np.float32)
    xo_full = np.concatenate([np.asarray(i["x"], np.float32).T, ones], 0).astype(np.float16)
    co_full = np.concatenate([np.asarray(i["c"], np.float32).T, ones], 0).astype(np.float16)
    epsT = eps.T.astype(np.float16)

    in_maps = []
    for ci in range(NCORES):
        s = slice(ci * BC, (ci + 1) * BC)
        m = dict(wmap)
        m["xo"] = np.ascontiguousarray(xo_full[:, s])
        m["co"] = np.ascontiguousarray(co_full[:, s])
        m["epsT"] = np.ascontiguousarray(epsT[:, s])
        in_maps.append(m)

    res = run_bass_kernel_spmd(nc, in_maps, core_ids=list(range(NCORES)))
    LAST_RESULTS = res

    out = np.empty((B, F), np.float32)
    mu = np.empty((B, L), np.float32)
    lv = np.empty((B, L), np.float32)
    for ci in range(NCORES):
        s = slice(ci * BC, (ci + 1) * BC)
        r = res.results[ci]
        out[s] = r["outT"].T
        mu[s] = r["mvT"][0:L].T
        lv[s] = r["mvT"][L : 2 * L].T
    return out, mu, lv


# revision 40
# speedup vs baseline: 1.1436x; 1.0328x over previous
"""Trainium2 Bass kernel for CustomPoseMixtureVAE (moe_routing).

Strategy: data-parallel over batch across 8 NeuronCores (256 rows/core),
all weights replicated, no collectives. Activations kept feature-major
[feat, batch] on-chip so every linear is a single PSUM-accumulated GEMM
chain with the (host-pretransposed) weight as the stationary operand.

The expert mixture  out = einsum('be,bi,eio->bo', coeff, inp, W)  is
computed as ONE GEMM over K = E*in using per-expert coefficient-scaled
inputs stacked along K:  out[b,o] = sum_{e,i} (coeff[b,e]*inp[b,i]) W[e,i,o].
Coefficient rows are broadcast across partitions with one-hot selector
matmuls on the PE; scaling is elementwise on DVE. The mixed bias
(coeff @ b_e) is folded in as an extra K-tile whose rhs is coeffT and
whose lhsT is the bias matrix.

ELU(x) = max(x, min(exp(x),1) - 1)   (exp monotonic => exp(min(x,0)) =
min(exp(x),1)); exp on ScalarE, the rest on Vector/GpSimd engines.
"""

import numpy as np
from contextlib import ExitStack

import concourse.bass as bass
import concourse.bacc as bacc
import concourse.tile as tile
import concourse.mybir as mybir
from concourse.bass_utils import run_bass_kernel_spmd

F16 = mybir.dt.float16
F32 = mybir.dt.float32
AOP = mybir.AluOpType
AF = mybir.ActivationFunctionType

B = 2048
NCORES = 8
BC = B // NCORES          # 256 batch rows per core
F = 267                   # frame size
L = 32                    # latent
H = 256                   # hidden
E = 8                     # experts
G = 64                    # gate hidden
IN0 = L + F               # 299
IN1 = L + H               # 288

LAST_RESULTS = None       # BassKernelResults of the most recent run
_CACHE = {}


def _eps42():
    """eps = jax.random.normal(key(42), (B, L)) exactly as the reference
    computes it, on the default jax backend (PRNG lowerings differ between
    backends, so we must mirror the reference's code path, not hardcode)."""
    if "eps" not in _CACHE:
        import jax
        import jax.numpy as jnp

        _CACHE["eps"] = np.asarray(
            jax.random.normal(jax.random.key(42), (B, L), jnp.float32)
        )
    return _CACHE["eps"]


def _pad_rows(a, rows):
    out = np.zeros((rows, a.shape[1]), a.dtype)
    out[: a.shape[0]] = a
    return out


def _bcast_ap(ap2d, reps):
    """[P, N] AP -> [P, reps, N] AP with a step-0 middle dim (operand bcast)."""
    return bass.AP(
        tensor=ap2d.tensor,
        offset=ap2d.offset,
        ap=[ap2d.ap[0], [0, reps], ap2d.ap[1]],
    )


def _build_program():
    nc = bacc.Bacc("TRN2")
    d = {}

    def din(name, shape, dt=F16):
        d[name] = nc.dram_tensor(name, shape, dt, kind="ExternalInput").ap()

    def dout(name, shape, dt=F32):
        d[name] = nc.dram_tensor(name, shape, dt, kind="ExternalOutput").ap()

    # per-core activations ([feat, batch] fp16; xo/co carry a trailing ones row)
    din("xo", [268, BC])
    din("co", [268, BC])
    din("epsT", [L, BC])
    # replicated weights (host-restacked, see kernel())
    din("sel", [8, 9, 128])
    din("ident", [128, 128], F32)
    din("fc1w", [6 * 128, 256])
    din("fc2w", [5 * 128, 256])
    din("mvw", [5 * 128, 64])
    din("g0w", [4 * 128, 64])
    din("g1w", [128, 64])
    din("g1wb", [1, 64])
    din("g2w", [128, 8])
    din("g2wb", [1, 8])
    din("w0st", [21 * 128, 256])
    din("w1st", [19 * 128, 256])
    din("w2st", [19 * 128, 267])
    dout("outT", [F, BC])
    dout("mvT", [2 * L, BC])

    with tile.TileContext(nc) as tc, ExitStack() as ctx:
        sb = ctx.enter_context(tc.tile_pool(name="sb", bufs=1))
        ps_big = ctx.enter_context(tc.tile_pool(name="ps_big", bufs=2, space="PSUM"))
        ps_sm = ctx.enter_context(tc.tile_pool(name="ps_sm", bufs=3, space="PSUM"))
        ps_bc = ctx.enter_context(tc.tile_pool(name="ps_bc", bufs=2, space="PSUM"))

        def sbt(tag, shape, dt=F16):
            return sb.tile(shape, dt, tag=tag, name=tag)

        # ---- input / weight DMAs -------------------------------------
        def dma_tiled(dst, src, nk, cols, eng=None):
            """DRAM [nk*128, cols] -> SBUF [128, nk, cols]."""
            eng = eng or nc.sync
            eng.dma_start(
                dst[:, 0:nk, :],
                src[0 : nk * 128, :].rearrange("(k p) m -> p k m", p=128),
            )

        # Spread DMAs over all five engines' queues so they pull in parallel,
        # first-needed tensors first on each queue.
        engs = [nc.sync, nc.scalar, nc.gpsimd]

        # DMA ordering is bandwidth-critical (~4.7MB/core at the shared HBM):
        # first-needed tensors first, split across both HWDGE queues; tiny
        # constants ride the SWDGE (gpsimd) queue.
        xo = sbt("xo", [128, 3, BC])
        nc.sync.dma_start(xo[:, 0:2, :], d["xo"][0:256, :].rearrange("(k p) n -> p k n", p=128))
        nc.gpsimd.dma_start(xo[0:12, 2, :], d["xo"][256:268, :])
        co = sbt("co", [128, 3, BC])
        nc.scalar.dma_start(co[:, 0:2, :], d["co"][0:256, :].rearrange("(k p) n -> p k n", p=128))
        nc.gpsimd.dma_start(co[0:12, 2, :], d["co"][256:268, :])
        fc1w = sbt("fc1w", [128, 6, 256])
        nc.sync.dma_start(fc1w[:, 0:3, :], d["fc1w"][0:384, :].rearrange("(k p) m -> p k m", p=128))
        nc.scalar.dma_start(fc1w[:, 3:6, :], d["fc1w"][384:768, :].rearrange("(k p) m -> p k m", p=128))
        fc2w = sbt("fc2w", [128, 5, 256])
        nc.sync.dma_start(fc2w[:, 0:3, :], d["fc2w"][0:384, :].rearrange("(k p) m -> p k m", p=128))
        nc.scalar.dma_start(fc2w[:, 3:5, :], d["fc2w"][384:640, :].rearrange("(k p) m -> p k m", p=128))
        eps = sbt("eps", [L, BC])
        nc.gpsimd.dma_start(eps[:], d["epsT"][:])
        sel = sbt("sel", [8, 9, 128])
        nc.gpsimd.dma_start(sel[:], d["sel"][:])
        ident = sbt("ident", [128, 128], F32)
        nc.gpsimd.dma_start(ident[:], d["ident"][:])

        # touch the exp table now so ACT_TABLE_LOAD overlaps the input DMAs.
        # NOTE: DMA issue occupies the issuing engine's FIFO for ~650ns per
        # dma_start — the Scalar engine must stay free for ACT compute, so
        # everything below goes on sync (idle) or the gpsimd SWDGE.
        scratch1 = sbt("scratch1", [1, 8])
        nc.scalar.activation(scratch1[:], sel[0:1, 8, 0:8], AF.Exp)

        mvw = sbt("mvw", [128, 5, 64])
        dma_tiled(mvw, d["mvw"], 5, 64, eng=nc.gpsimd)
        g0w = sbt("g0w", [128, 4, 64])
        dma_tiled(g0w, d["g0w"], 4, 64, eng=nc.gpsimd)
        g1w = sbt("g1w", [128, 64])
        nc.gpsimd.dma_start(g1w[:], d["g1w"][:])
        g1wb = sbt("g1wb", [1, 64])
        nc.gpsimd.dma_start(g1wb[:], d["g1wb"][:])
        g2w = sbt("g2w", [128, 8])
        nc.gpsimd.dma_start(g2w[:], d["g2w"][:])
        g2wb = sbt("g2wb", [1, 8])
        nc.gpsimd.dma_start(g2wb[:], d["g2wb"][:])

        # decoder weights: few large transfers, all issued by the idle sync
        # engine; the dynamic HWDGE pool streams them in the background
        w0 = sbt("w0", [128, 21, 256])
        w1 = sbt("w1", [128, 19, 256])
        w2 = sbt("w2", [128, 19, 267])
        for (wt, src, nk) in ((w0, d["w0st"], 21), (w1, d["w1st"], 19), (w2, d["w2st"], 19)):
            mid = (nk + 1) // 2
            for lo, hi in ((0, mid), (mid, nk)):
                nc.sync.dma_start(
                    wt[:, lo:hi, :],
                    src[lo * 128 : hi * 128, :].rearrange("(k p) m -> p k m", p=128),
                )

        # L0 c-tail scaled tiles: zero the 21-row pads once, up front
        s0t = sbt("s0t", [128, 2, BC])
        nc.vector.memset(s0t[:], 0.0)

        # ---- helpers -------------------------------------------------
        def elu(pp, width, out_ap, tagbase):
            """ELU(x) = relu(x) + (min(exp(x),1) - 1), x = psum pp [P, width]."""
            p = pp.shape[0]
            ex = sbt(tagbase + "_e", [p, width])
            nc.scalar.activation(ex[:], pp, AF.Exp)
            tt = sbt(tagbase + "_t", [p, width])
            nc.vector.tensor_scalar(tt[:], ex[:], 1.0, 1.0, AOP.min, AOP.subtract)
            nc.vector.scalar_tensor_tensor(out_ap, pp, 0.0, tt[:], AOP.max, AOP.add)

        # ---- encoder -------------------------------------------------
        with nc.named_scope("enc1"):
            ph1 = ps_big.tile([128, 512], F32, tag="pbig", name="pbig")
            enc1_rhs = [
                (xo[:, 0, :], slice(0, 128)),
                (xo[:, 1, :], slice(0, 128)),
                (xo[0:12, 2, :], slice(0, 12)),
                (co[:, 0, :], slice(0, 128)),
                (co[:, 1, :], slice(0, 128)),
                (co[0:12, 2, :], slice(0, 12)),
            ]
            for m in range(2):
                n = len(enc1_rhs)
                for i, (rh, krows) in enumerate(enc1_rhs):
                    nc.tensor.matmul(
                        ph1[:, m * BC : (m + 1) * BC],
                        fc1w[krows, i, m * 128 : (m + 1) * 128],
                        rh,
                        start=(i == 0),
                        stop=(i == n - 1),
                    )
            h1 = sbt("h1", [128, 2, BC])
            elu(ph1[:], 512, h1[:].rearrange("p k n -> p (k n)"), "eh1")

        with nc.named_scope("enc2"):
            ph2 = ps_big.tile([128, 512], F32, tag="pbig", name="pbig")
            enc2_rhs = [
                (xo[:, 0, :], slice(0, 128)),
                (xo[:, 1, :], slice(0, 128)),
                (xo[0:12, 2, :], slice(0, 12)),
                (h1[:, 0, :], slice(0, 128)),
                (h1[:, 1, :], slice(0, 128)),
            ]
            for m in range(2):
                n = len(enc2_rhs)
                for i, (rh, krows) in enumerate(enc2_rhs):
                    nc.tensor.matmul(
                        ph2[:, m * BC : (m + 1) * BC],
                        fc2w[krows, i, m * 128 : (m + 1) * 128],
                        rh,
                        start=(i == 0),
                        stop=(i == n - 1),
                    )
            h2 = sbt("h2", [128, 2, BC])
            elu(ph2[:], 512, h2[:].rearrange("p k n -> p (k n)"), "eh2")

        with nc.named_scope("muvar"):
            pmv = ps_sm.tile([64, BC], F32, tag="psm", name="psm")
            mv_rhs = [
                (xo[:, 0, :], slice(0, 128)),
                (xo[:, 1, :], slice(0, 128)),
                (xo[0:12, 2, :], slice(0, 12)),
                (h2[:, 0, :], slice(0, 128)),
                (h2[:, 1, :], slice(0, 128)),
            ]
            n = len(mv_rhs)
            for i, (rh, krows) in enumerate(mv_rhs):
                nc.tensor.matmul(pmv[:], mvw[krows, i, 0:64], rh, start=(i == 0), stop=(i == n - 1))
            # z = mu + eps * exp(0.5*logvar)
            stdt = sbt("stdt", [L, BC])
            nc.scalar.activation(stdt[:], pmv[32:64, :], AF.Exp, scale=0.5)
            zt = sbt("zt", [L, BC])
            nc.vector.tensor_mul(zt[:], eps[:], stdt[:])
            z = sbt("z", [L, BC])
            nc.vector.tensor_add(z[:], zt[:], pmv[0:32, :])
            mvf = sbt("mvf", [64, BC], F32)
            nc.scalar.copy(mvf[:], pmv[:])                 # fp32 mu/logvar out
            nc.sync.dma_start(d["mvT"][:], mvf[:])

        # ---- gate ----------------------------------------------------
        # Gate ELUs are folded into the next matmul:  W@elu(x) = W@relu(x)
        # + W@t with t = min(exp(x),1)-1, so the 64-row relu/t halves stack
        # into one 128-row K-tile (weights duplicated host-side) and the DVE
        # max-combine drops off the serial chain.
        with nc.named_scope("gate"):
            onesr = sbt("onesr", [1, BC])
            nc.vector.memset(onesr[:], 1.0)
            pg0 = ps_sm.tile([64, BC], F32, tag="psm", name="psm")
            # z-free tiles first so the PE can start before z is ready
            g0_rhs = [
                (co[:, 0, :], slice(0, 128)),
                (co[:, 1, :], slice(0, 128)),
                (co[0:12, 2, :], slice(0, 12)),
                (z[:], slice(0, 32)),
            ]
            n = len(g0_rhs)
            for i, (rh, krows) in enumerate(g0_rhs):
                nc.tensor.matmul(pg0[:], g0w[krows, i, 0:64], rh, start=(i == 0), stop=(i == n - 1))

            def gate_relu_t(pp, tagbase):
                ga = sbt(tagbase + "_a", [128, BC])
                ex = sbt(tagbase + "_e", [64, BC])
                nc.scalar.activation(ex[:], pp, AF.Exp)
                nc.scalar.activation(ga[0:64, :], pp, AF.Relu)
                nc.vector.tensor_scalar(ga[64:128, :], ex[:], 1.0, 1.0, AOP.min, AOP.subtract)
                return ga

            g1in = gate_relu_t(pg0[:], "eg0")
            pg1 = ps_sm.tile([64, BC], F32, tag="psm", name="psm")
            nc.tensor.matmul(pg1[:], g1w[:, 0:64], g1in[:], start=True, stop=False)
            nc.tensor.matmul(pg1[:], g1wb[0:1, 0:64], onesr[:], start=False, stop=True)

            g2in = gate_relu_t(pg1[:], "eg1")
            plg = ps_sm.tile([8, BC], F32, tag="psm", name="psm")
            nc.tensor.matmul(plg[:], g2w[:, 0:8], g2in[:], start=True, stop=False)
            nc.tensor.matmul(plg[:], g2wb[0:1, 0:8], onesr[:], start=False, stop=True)

            # Softmax without the normalizer on the critical path: broadcast
            # UN-normalized exp(logits) per expert right away (selector
            # matmuls) and scale each decoder layer's psum output by the
            # per-sample 1/sum afterwards (mixing is linear in the coeffs).
            expE = sbt("expE", [8, BC])
            nc.scalar.activation(expE[:], plg[:], AF.Exp)

            bcastE = sbt("bcastE", [128, 8, BC])
            for i in range(4):
                pbc = ps_bc.tile([128, 512], F32, tag="pbc", name="pbc")
                nc.tensor.matmul(pbc[:, 0:BC], sel[:, 2 * i, :], expE[:], start=True, stop=True)
                nc.tensor.matmul(pbc[:, BC:512], sel[:, 2 * i + 1, :], expE[:], start=True, stop=True)
                if i % 2 == 0:
                    nc.vector.tensor_copy(bcastE[:, 2 * i : 2 * i + 2, :].rearrange("p k n -> p (k n)"), pbc[:])
                else:
                    nc.scalar.copy(bcastE[:, 2 * i : 2 * i + 2, :].rearrange("p k n -> p (k n)"), pbc[:])

            # 1/sum pipeline, off the critical path. Sum batch-on-partitions
            # (so the DVE iterative divide runs 2 elem/lane on 128 lanes),
            # transpose back via identity matmuls, then broadcast to 128 rows.
            pst = ps_sm.tile([128, 2], F32, tag="psm", name="psm")
            nc.tensor.matmul(pst[:, 0:1], expE[0:8, 0:128], sel[0:8, 8, 0:1], start=True, stop=True)
            nc.tensor.matmul(pst[:, 1:2], expE[0:8, 128:256], sel[0:8, 8, 0:1], start=True, stop=True)
            rst = sbt("rst", [128, 2], F32)
            with nc.allow_low_precision(reason="softmax denominators are well-conditioned"):
                nc.vector.reciprocal(rst[:], pst[:])
            precS = ps_sm.tile([1, BC], F32, tag="psm", name="psm")
            nc.tensor.matmul(precS[0:1, 0:128], rst[:, 0:1], ident[:], start=True, stop=True)
            nc.tensor.matmul(precS[0:1, 128:256], rst[:, 1:2], ident[:], start=True, stop=True)
            recS = sbt("recS", [1, BC])
            nc.scalar.copy(recS[:], precS[:])
            prsb = ps_sm.tile([128, BC], F32, tag="psm", name="psm")
            nc.tensor.matmul(prsb[:], sel[0:1, 8, 0:128], recS[:], start=True, stop=True)
            rsb = sbt("rsb", [128, BC])
            nc.vector.tensor_copy(rsb[:], prsb[:])

        # ---- decoder -------------------------------------------------
        # z-part of the scaled stack (shared by all 3 layers)
        with nc.named_scope("zstack"):
            zs = sbt("zs", [128, 2, BC])
            for e in range(E):
                nc.vector.tensor_tensor(
                    zs[32 * (e % 4) : 32 * (e % 4) + 32, e // 4, :],
                    z[:],
                    bcastE[0:32, e, :],
                    AOP.mult,
                )

        def scaled_stack(name, src3, nsub):
            """Per-expert coefficient-scaled copies of src3 [128, nsub, BC]."""
            st = sbt(name, [128, E * nsub, BC])
            for e in range(E):
                nc.vector.tensor_tensor(
                    st[:, e * nsub : (e + 1) * nsub, :],
                    src3,
                    _bcast_ap(bcastE[:, e, :], nsub),
                    AOP.mult,
                )
            return st

        def decoder_layer(name, wt, stack_tiles, psum_w, mtiles, out_elu):
            """stack_tiles: list of (rhs_ap, krows) aligned with wt subtiles.
            The psum holds the S-scaled mixture; normalize by rsb (1/S per
            sample) on eviction, then ELU in SBUF if requested."""
            with nc.named_scope(name):
                pl = ps_big.tile([128, 512], F32, tag="pbig", name="pbig") if psum_w == 512 else None
                outs = []
                for mi, (mlo, mhi) in enumerate(mtiles):
                    if mhi - mlo == 128 and pl is not None:
                        pap = pl[:, mi * BC : (mi + 1) * BC]
                    else:
                        ptail = ps_sm.tile([mhi - mlo, BC], F32, tag="psm", name="psm")
                        pap = ptail[:]
                        outs.append(ptail)
                    n = len(stack_tiles)
                    for i, (rh, krows) in enumerate(stack_tiles):
                        nc.tensor.matmul(
                            pap,
                            wt[krows, i, mlo:mhi],
                            rh,
                            start=(i == 0),
                            stop=(i == n - 1),
                        )
                if out_elu is not None:
                    y = sbt("y" + name, [128, 512])
                    nc.vector.tensor_tensor(
                        y[:].rearrange("p (k n) -> p k n", n=BC),
                        pl[:].rearrange("p (k n) -> p k n", n=BC),
                        _bcast_ap(rsb[:], 2),
                        AOP.mult,
                    )
                    ex = sbt("e" + name + "_e", [128, 512])
                    nc.scalar.activation(ex[:], y[:], AF.Exp)
                    tt = sbt("e" + name + "_t", [128, 512])
                    nc.vector.tensor_scalar(tt[:], ex[:], 1.0, 1.0, AOP.min, AOP.subtract)
                    nc.vector.tensor_tensor(out_elu[:].rearrange("p k n -> p (k n)"), y[:], tt[:], AOP.max)
                return pl, outs

        with nc.named_scope("l0scale"):
            s0 = scaled_stack("s0", co[:, 0:2, :], 2)
            # c tail rows (11 per expert, padded to 32-partition blocks: compute
            # engines require 32-aligned partition bases)
            for e in range(E):
                nc.vector.tensor_tensor(
                    s0t[32 * (e % 4) : 32 * (e % 4) + 11, e // 4, :],
                    co[0:11, 2, :],
                    bcastE[0:11, e, :],
                    AOP.mult,
                )

        l0_tiles = [(s0[:, j, :], slice(0, 128)) for j in range(16)]
        l0_tiles += [(s0t[:, 0, :], slice(0, 128)), (s0t[:, 1, :], slice(0, 128))]
        l0_tiles += [(zs[:, 0, :], slice(0, 128)), (zs[:, 1, :], slice(0, 128)), (expE[:], slice(0, 8))]
        l0o = sbt("l0o", [128, 2, BC])
        decoder_layer("l0", w0, l0_tiles, 512, [(0, 128), (128, 256)], l0o)

        with nc.named_scope("l1scale"):
            s1 = scaled_stack("s1", l0o[:, 0:2, :], 2)
        l1_tiles = [(s1[:, j, :], slice(0, 128)) for j in range(16)]
        l1_tiles += [(zs[:, 0, :], slice(0, 128)), (zs[:, 1, :], slice(0, 128)), (expE[:], slice(0, 8))]
        l1o = sbt("l1o", [128, 2, BC])
        decoder_layer("l1", w1, l1_tiles, 512, [(0, 128), (128, 256)], l1o)

        with nc.named_scope("l2scale"):
            s2 = scaled_stack("s2", l1o[:, 0:2, :], 2)
        l2_tiles = [(s2[:, j, :], slice(0, 128)) for j in range(16)]
        l2_tiles += [(zs[:, 0, :], slice(0, 128)), (zs[:, 1, :], slice(0, 128)), (expE[:], slice(0, 8))]
        pl2, tails = decoder_layer(
            "l2", w2, l2_tiles, 512, [(0, 128), (128, 256), (256, 267)], None
        )

        with nc.named_scope("out"):
            out0 = sbt("out0", [128, 512], F32)
            nc.vector.tensor_mul(out0[:, 0:BC], pl2[:, 0:BC], rsb[:])
            nc.sync.dma_start(d["outT"][0:128, :], out0[:, 0:BC])
            nc.vector.tensor_mul(out0[:, BC:512], pl2[:, BC:512], rsb[:])
            nc.scalar.dma_start(d["outT"][128:256, :], out0[:, BC:512])
            out2 = sbt("out2", [11, BC], F32)
            nc.vector.tensor_mul(out2[:], tails[0][:], rsb[0:11, :])
            nc.sync.dma_start(d["outT"][256:267, :], out2[:])

    nc.compile()
    return nc


def _host_weights(i):
    """Restack/transpose/cast all weights for the device layout."""
    f16 = np.float16

    def t(a):
        return np.asarray(a, np.float32).T  # [in, out]

    W1t = t(i["fc1_w"])  # [534, 256]
    fc1 = np.concatenate(
        [
            W1t[0:128],
            W1t[128:256],
            _pad_rows(np.concatenate([W1t[256:267], i["fc1_b"][None, :]], 0), 128),
            W1t[267:395],
            W1t[395:523],
            _pad_rows(W1t[523:534], 128),
        ],
        0,
    )
    W2t = t(i["fc2_w"])  # [523, 256]
    fc2 = np.concatenate(
        [
            W2t[0:128],
            W2t[128:256],
            _pad_rows(np.concatenate([W2t[256:267], i["fc2_b"][None, :]], 0), 128),
            W2t[267:395],
            W2t[395:523],
        ],
        0,
    )
    Wmv = np.concatenate([t(i["mu_w"]), t(i["lv_w"])], 1)  # [523, 64]
    bmv = np.concatenate([i["mu_b"], i["lv_b"]])[None, :]
    mv = np.concatenate(
        [
            Wmv[0:128],
            Wmv[128:256],
            _pad_rows(np.concatenate([Wmv[256:267], bmv], 0), 128),
            Wmv[267:395],
            Wmv[395:523],
        ],
        0,
    )
    G0 = t(i["g0_w"])  # [299, 64]
    g0 = np.concatenate(
        [
            G0[32:160],
            G0[160:288],
            _pad_rows(np.concatenate([G0[288:299], i["g0_b"][None, :]], 0), 128),
            _pad_rows(G0[0:32], 128),
        ],
        0,
    )
    g1 = np.concatenate([t(i["g1_w"]), t(i["g1_w"])], 0)  # relu-half + t-half
    g2 = np.concatenate([t(i["g2_w"]), t(i["g2_w"])], 0)

    def dec_stack(w, b):
        w = np.asarray(w, np.float32)  # [E, in, out]
        parts = []
        for e in range(E):
            parts.append(w[e, 32:160])
            parts.append(w[e, 160:288])
        if w.shape[1] == IN0:  # layer 0: c tail rows, 32-row block per expert
            for g in range(2):
                parts.append(
                    np.concatenate(
                        [_pad_rows(w[e, 288:299], 32) for e in range(4 * g, 4 * g + 4)], 0
                    )
                )
        parts.append(np.concatenate([w[e, 0:32] for e in range(4)], 0))
        parts.append(np.concatenate([w[e, 0:32] for e in range(4, 8)], 0))
        parts.append(_pad_rows(np.asarray(b, np.float32), 128))
        return np.concatenate(parts, 0)

    w0st = dec_stack(i["w0"], i["b0"])
    w1st = dec_stack(i["w1"], i["b1"])
    w2st = dec_stack(i["w2"], i["b2"])

    sel = np.zeros((8, 9, 128), np.float32)
    for e in range(E):
        sel[e, e, :] = 1.0
    sel[:, 8, :] = 1.0

    return {
        "ident": np.eye(128, dtype=np.float32),
        "sel": sel.astype(f16),
        "fc1w": fc1.astype(f16),
        "fc2w": fc2.astype(f16),
        "mvw": mv.astype(f16),
        "g0w": g0.astype(f16),
        "g1w": g1.astype(f16),
        "g1wb": np.asarray(i["g1_b"], np.float32)[None, :].astype(f16),
        "g2w": g2.astype(f16),
        "g2wb": np.asarray(i["g2_b"], np.float32)[None, :].astype(f16),
        "w0st": w0st.astype(f16),
        "w1st": w1st.astype(f16),
        "w2st": w2st.astype(f16),
    }


def kernel(**inputs):
    global LAST_RESULTS
    if "nc" not in _CACHE:
        _CACHE["nc"] = _build_program()
    nc = _CACHE["nc"]

    i = {k: np.asarray(v) for k, v in inputs.items()}
    eps = _eps42()
    wmap = _host_weights(i)

    ones = np.ones((1, B), np.float32)
    xo_full = np.concatenate([np.asarray(i["x"], np.float32).T, ones], 0).astype(np.float16)
    co_full = np.concatenate([np.asarray(i["c"], np.float32).T, ones], 0).astype(np.float16)
    epsT = eps.T.astype(np.float16)

    in_maps = []
    for ci in range(NCORES):
        s = slice(ci * BC, (ci + 1) * BC)
        m = dict(wmap)
        m["xo"] = np.ascontiguousarray(xo_full[:, s])
        m["co"] = np.ascontiguousarray(co_full[:, s])
        m["epsT"] = np.ascontiguousarray(epsT[:, s])
        in_maps.append(m)

    res = run_bass_kernel_spmd(nc, in_maps, core_ids=list(range(NCORES)))
    LAST_RESULTS = res

    out = np.empty((B, F), np.float32)
    mu = np.empty((B, L), np.float32)
    lv = np.empty((B, L), np.float32)
    for ci in range(NCORES):
        s = slice(ci * BC, (ci + 1) * BC)
        r = res.results[ci]
        out[s] = r["outT"].T
        mu[s] = r["mvT"][0:L].T
        lv[s] = r["mvT"][L : 2 * L].T
    return out, mu, lv


# revision 41
# speedup vs baseline: 1.2332x; 1.0783x over previous
"""Trainium2 Bass kernel for CustomPoseMixtureVAE (moe_routing).

Strategy: data-parallel over batch across 8 NeuronCores (256 rows/core),
all weights replicated, no collectives. Activations kept feature-major
[feat, batch] on-chip so every linear is a single PSUM-accumulated GEMM
chain with the (host-pretransposed) weight as the stationary operand;
biases are folded in as extra K-rows against ones/coefficient rows.

The expert mixture  out = einsum('be,bi,eio->bo', coeff, inp, W)  is one
GEMM over K = E*in of per-expert coefficient-scaled inputs stacked along
K. The softmax normalizer stays OFF the critical path: un-normalized
exp(logits) rows are broadcast across partitions with one-hot selector
matmuls and the per-sample 1/sum is multiplied into each layer's psum
output (the mixture is linear in the coefficients).

To keep the PE's HAM clock-gate warm (2.4 GHz) the decoder is scheduled
as one continuous matmul stream: each layer's two M-half accumulation
chains write separate PSUM banks, the half-0 normalize+ELU+rescale runs
on DVE/ACT while the PE is still on half-1, and the next layer's K-feed
is ordered so half-1-dependent tiles come last.

ELU(x) = relu(x) + (min(exp(x),1) - 1)  (exp monotonic); the gate MLP
folds its ELUs into the next matmul via W@elu(x) = W@relu(x) - W'@relu(1
- exp(x)) with W' a host-negated copy.
"""

import numpy as np
from contextlib import ExitStack

import concourse.bass as bass
import concourse.bacc as bacc
import concourse.tile as tile
import concourse.mybir as mybir
from concourse.bass_utils import run_bass_kernel_spmd

F16 = mybir.dt.float16
F32 = mybir.dt.float32
AOP = mybir.AluOpType
AF = mybir.ActivationFunctionType

B = 2048
NCORES = 8
BC = B // NCORES          # 256 batch rows per core
F = 267                   # frame size
L = 32                    # latent
H = 256                   # hidden
E = 8                     # experts
G = 64                    # gate hidden
IN0 = L + F               # 299
IN1 = L + H               # 288

LAST_RESULTS = None       # BassKernelResults of the most recent run
_CACHE = {}


def _eps42():
    """eps = jax.random.normal(key(42), (B, L)) exactly as the reference
    computes it, on the default jax backend (PRNG lowerings differ between
    backends, so we must mirror the reference's code path, not hardcode)."""
    if "eps" not in _CACHE:
        import jax
        import jax.numpy as jnp

        _CACHE["eps"] = np.asarray(
            jax.random.normal(jax.random.key(42), (B, L), jnp.float32)
        )
    return _CACHE["eps"]


def _pad_rows(a, rows):
    out = np.zeros((rows, a.shape[1]), a.dtype)
    out[: a.shape[0]] = a
    return out


def _bcast_ap(ap2d, reps):
    """[P, N] AP -> [P, reps, N] AP with a step-0 middle dim (operand bcast)."""
    return bass.AP(
        tensor=ap2d.tensor,
        offset=ap2d.offset,
        ap=[ap2d.ap[0], [0, reps], ap2d.ap[1]],
    )


def _build_program():
    nc = bacc.Bacc("TRN2")
    d = {}

    def din(name, shape, dt=F16):
        d[name] = nc.dram_tensor(name, shape, dt, kind="ExternalInput").ap()

    def dout(name, shape, dt=F32):
        d[name] = nc.dram_tensor(name, shape, dt, kind="ExternalOutput").ap()

    # per-core activations ([feat, batch] fp16; xo/co carry a trailing ones row)
    din("xo", [268, BC])
    din("co", [268, BC])
    din("epsT", [L, BC])
    # replicated weights (host-restacked, see kernel())
    din("sel", [8, 9, 128])
    din("ident", [128, 128], F32)
    din("fc1w", [6 * 128, 256])
    din("fc2w", [5 * 128, 256])
    din("mvw", [5 * 128, 64])
    din("g0w", [4 * 128, 64])
    din("g1w", [128, 64])
    din("g1wb", [1, 64])
    din("g2w", [128, 8])
    din("g2wb", [1, 8])
    din("w0st", [21 * 128, 256])
    din("w1st", [19 * 128, 256])
    din("w2st", [19 * 128, 267])
    dout("outT", [F, BC])
    dout("mvT", [2 * L, BC])

    with tile.TileContext(nc) as tc, ExitStack() as ctx:
        sb = ctx.enter_context(tc.tile_pool(name="sb", bufs=1))
        # separate banks per M-half so DVE can read half-0 while the PE
        # accumulates half-1 (same-bank PE-W/DVE-R would serialize)
        ps_big = ctx.enter_context(tc.tile_pool(name="ps_big", bufs=4, space="PSUM"))
        ps_sm = ctx.enter_context(tc.tile_pool(name="ps_sm", bufs=2, space="PSUM"))
        ps_bc = ctx.enter_context(tc.tile_pool(name="ps_bc", bufs=2, space="PSUM"))

        def sbt(tag, shape, dt=F16):
            return sb.tile(shape, dt, tag=tag, name=tag)

        def psb():
            return ps_big.tile([128, BC], F32, tag="pbig", name="pbig")

        # ---- input / weight DMAs -------------------------------------
        # DMA issue occupies the issuing engine's FIFO (~650ns per
        # dma_start): scalar only gets what it must, bulk goes on sync,
        # tiny constants on the gpsimd SWDGE.
        def dma_tiled(dst, src, nk, cols, eng):
            eng.dma_start(
                dst[:, 0:nk, :],
                src[0 : nk * 128, :].rearrange("(k p) m -> p k m", p=128),
            )

        xo = sbt("xo", [128, 3, BC])
        nc.sync.dma_start(xo[:, 0:2, :], d["xo"][0:256, :].rearrange("(k p) n -> p k n", p=128))
        nc.gpsimd.dma_start(xo[0:12, 2, :], d["xo"][256:268, :])
        co = sbt("co", [128, 3, BC])
        nc.scalar.dma_start(co[:, 0:2, :], d["co"][0:256, :].rearrange("(k p) n -> p k n", p=128))
        nc.gpsimd.dma_start(co[0:12, 2, :], d["co"][256:268, :])
        fc1w = sbt("fc1w", [128, 6, 256])
        nc.sync.dma_start(fc1w[:, 0:3, :], d["fc1w"][0:384, :].rearrange("(k p) m -> p k m", p=128))
        nc.scalar.dma_start(fc1w[:, 3:6, :], d["fc1w"][384:768, :].rearrange("(k p) m -> p k m", p=128))
        fc2w = sbt("fc2w", [128, 5, 256])
        nc.sync.dma_start(fc2w[:, 0:3, :], d["fc2w"][0:384, :].rearrange("(k p) m -> p k m", p=128))
        nc.scalar.dma_start(fc2w[:, 3:5, :], d["fc2w"][384:640, :].rearrange("(k p) m -> p k m", p=128))
        eps = sbt("eps", [L, BC])
        nc.gpsimd.dma_start(eps[:], d["epsT"][:])
        sel = sbt("sel", [8, 9, 128])
        nc.gpsimd.dma_start(sel[:], d["sel"][:])
        ident = sbt("ident", [128, 128], F32)
        nc.gpsimd.dma_start(ident[:], d["ident"][:])

        # touch the exp table now so ACT_TABLE_LOAD overlaps the input DMAs
        scratch1 = sbt("scratch1", [1, 8])
        nc.scalar.activation(scratch1[:], sel[0:1, 8, 0:8], AF.Exp)

        mvw = sbt("mvw", [128, 5, 64])
        dma_tiled(mvw, d["mvw"], 5, 64, nc.gpsimd)
        g0w = sbt("g0w", [128, 4, 64])
        dma_tiled(g0w, d["g0w"], 4, 64, nc.gpsimd)
        g1w = sbt("g1w", [128, 64])
        nc.gpsimd.dma_start(g1w[:], d["g1w"][:])
        g1wb = sbt("g1wb", [1, 64])
        nc.gpsimd.dma_start(g1wb[:], d["g1wb"][:])
        g2w = sbt("g2w", [128, 8])
        nc.gpsimd.dma_start(g2w[:], d["g2w"][:])
        g2wb = sbt("g2wb", [1, 8])
        nc.gpsimd.dma_start(g2wb[:], d["g2wb"][:])

        w0 = sbt("w0", [128, 21, 256])
        w1 = sbt("w1", [128, 19, 256])
        w2 = sbt("w2", [128, 19, 267])
        for (wt, src, nk) in ((w0, d["w0st"], 21), (w1, d["w1st"], 19), (w2, d["w2st"], 19)):
            mid = (nk + 1) // 2
            for lo, hi in ((0, mid), (mid, nk)):
                nc.sync.dma_start(
                    wt[:, lo:hi, :],
                    src[lo * 128 : hi * 128, :].rearrange("(k p) m -> p k m", p=128),
                )

        # L0 c-tail scaled tiles: zero the 21-row pads once, up front
        s0t = sbt("s0t", [128, 2, BC])
        nc.vector.memset(s0t[:], 0.0)

        # ---- helpers -------------------------------------------------
        def mm_chain(pap, kfeed, wt, mlo, mhi):
            n = len(kfeed)
            for i, (rh, krows) in enumerate(kfeed):
                nc.tensor.matmul(
                    pap, wt[krows, i, mlo:mhi], rh, start=(i == 0), stop=(i == n - 1)
                )

        def elu_half(pp, out_ap, tagbase):
            """ELU from psum half pp [P, BC] into fp16 out_ap."""
            p = pp.shape[0]
            ex = sbt(tagbase + "_e", [p, BC])
            nc.scalar.activation(ex[:], pp, AF.Exp)
            tt = sbt(tagbase + "_t", [p, BC])
            nc.vector.tensor_scalar(tt[:], ex[:], 1.0, 1.0, AOP.min, AOP.subtract)
            nc.vector.scalar_tensor_tensor(out_ap, pp, 0.0, tt[:], AOP.max, AOP.add)

        # ---- encoder -------------------------------------------------
        enc1_rhs = [
            (xo[:, 0, :], slice(0, 128)),
            (xo[:, 1, :], slice(0, 128)),
            (xo[0:12, 2, :], slice(0, 12)),
            (co[:, 0, :], slice(0, 128)),
            (co[:, 1, :], slice(0, 128)),
            (co[0:12, 2, :], slice(0, 12)),
        ]
        h1 = sbt("h1", [128, 2, BC])
        with nc.named_scope("enc1"):
            for m in range(2):
                pp = psb()
                mm_chain(pp[:], enc1_rhs, fc1w, m * 128, (m + 1) * 128)
                elu_half(pp[:], h1[:, m, :], f"eh1{m}")

        enc2_rhs = [
            (xo[:, 0, :], slice(0, 128)),
            (xo[:, 1, :], slice(0, 128)),
            (xo[0:12, 2, :], slice(0, 12)),
            (h1[:, 0, :], slice(0, 128)),
            (h1[:, 1, :], slice(0, 128)),
        ]
        h2 = sbt("h2", [128, 2, BC])
        with nc.named_scope("enc2"):
            for m in range(2):
                pp = psb()
                mm_chain(pp[:], enc2_rhs, fc2w, m * 128, (m + 1) * 128)
                elu_half(pp[:], h2[:, m, :], f"eh2{m}")

        with nc.named_scope("muvar"):
            mv_rhs = [
                (xo[:, 0, :], slice(0, 128)),
                (xo[:, 1, :], slice(0, 128)),
                (xo[0:12, 2, :], slice(0, 12)),
                (h2[:, 0, :], slice(0, 128)),
                (h2[:, 1, :], slice(0, 128)),
            ]
            pmv = ps_sm.tile([64, BC], F32, tag="psm", name="psm")
            mm_chain(pmv[:], mv_rhs, mvw, 0, 64)
            # z = mu + eps * exp(0.5*logvar)
            stdt = sbt("stdt", [L, BC])
            nc.scalar.activation(stdt[:], pmv[32:64, :], AF.Exp, scale=0.5)
            zt = sbt("zt", [L, BC])
            nc.vector.tensor_mul(zt[:], eps[:], stdt[:])
            z = sbt("z", [L, BC])
            nc.vector.tensor_add(z[:], zt[:], pmv[0:32, :])
            mvf = sbt("mvf", [64, BC], F32)
            nc.scalar.copy(mvf[:], pmv[:])
            nc.sync.dma_start(d["mvT"][:], mvf[:])

        # ---- gate ----------------------------------------------------
        # ELUs folded into the next matmul: W@elu(x) = W@relu(x) -
        # Wneg@relu(1-exp(x)), both halves stacked into one 128-row K-tile.
        with nc.named_scope("gate"):
            onesr = sbt("onesr", [1, BC])
            nc.vector.memset(onesr[:], 1.0)
            pg0 = ps_sm.tile([64, BC], F32, tag="psm", name="psm")
            g0_rhs = [
                (co[:, 0, :], slice(0, 128)),
                (co[:, 1, :], slice(0, 128)),
                (co[0:12, 2, :], slice(0, 12)),
                (z[:], slice(0, 32)),
            ]
            mm_chain(pg0[:], g0_rhs, g0w, 0, 64)

            def gate_relu_t(pp, tagbase):
                ga = sbt(tagbase + "_a", [128, BC])
                ex = sbt(tagbase + "_e", [64, BC])
                nc.scalar.activation(ex[:], pp, AF.Exp)
                nc.scalar.activation(ga[0:64, :], pp, AF.Relu)
                # relu(1 - exp(x)) = -(min(exp(x),1) - 1); sign absorbed by Wneg
                nc.scalar.activation(ga[64:128, :], ex[:], AF.Relu, bias=1.0, scale=-1.0)
                return ga

            g1in = gate_relu_t(pg0[:], "eg0")
            pg1 = ps_sm.tile([64, BC], F32, tag="psm", name="psm")
            nc.tensor.matmul(pg1[:], g1w[:, 0:64], g1in[:], start=True, stop=False)
            nc.tensor.matmul(pg1[:], g1wb[0:1, 0:64], onesr[:], start=False, stop=True)

            g2in = gate_relu_t(pg1[:], "eg1")
            plg = ps_sm.tile([8, BC], F32, tag="psm", name="psm")
            nc.tensor.matmul(plg[:], g2w[:, 0:8], g2in[:], start=True, stop=False)
            nc.tensor.matmul(plg[:], g2wb[0:1, 0:8], onesr[:], start=False, stop=True)

            # un-normalized softmax numerators, broadcast per expert
            expE = sbt("expE", [8, BC])
            nc.scalar.activation(expE[:], plg[:], AF.Exp)

            bcastE = sbt("bcastE", [128, 8, BC])
            for i in range(4):
                pbc = ps_bc.tile([128, 512], F32, tag="pbc", name="pbc")
                nc.tensor.matmul(pbc[:, 0:BC], sel[:, 2 * i, :], expE[:], start=True, stop=True)
                nc.tensor.matmul(pbc[:, BC:512], sel[:, 2 * i + 1, :], expE[:], start=True, stop=True)
                dst = bcastE[:, 2 * i : 2 * i + 2, :].rearrange("p k n -> p (k n)")
                if i % 2 == 0:
                    nc.vector.tensor_copy(dst, pbc[:])
                else:
                    nc.scalar.copy(dst, pbc[:])

            # 1/sum pipeline (fully off the critical path): sum with batch on
            # partitions so the DVE divide runs 2 elem/lane, transpose back
            # via identity matmuls, broadcast to 128 rows.
            pst = ps_sm.tile([128, 2], F32, tag="psm", name="psm")
            nc.tensor.matmul(pst[:, 0:1], expE[0:8, 0:128], sel[0:8, 8, 0:1], start=True, stop=True)
            nc.tensor.matmul(pst[:, 1:2], expE[0:8, 128:256], sel[0:8, 8, 0:1], start=True, stop=True)
            rst = sbt("rst", [128, 2], F32)
            with nc.allow_low_precision(reason="softmax denominators are well-conditioned"):
                nc.vector.reciprocal(rst[:], pst[:])
            precS = ps_sm.tile([1, BC], F32, tag="psm", name="psm")
            nc.tensor.matmul(precS[0:1, 0:128], rst[:, 0:1], ident[:], start=True, stop=True)
            nc.tensor.matmul(precS[0:1, 128:256], rst[:, 1:2], ident[:], start=True, stop=True)
            recS = sbt("recS", [1, BC])
            nc.scalar.copy(recS[:], precS[:])
            prsb = ps_sm.tile([128, BC], F32, tag="psm", name="psm")
            nc.tensor.matmul(prsb[:], sel[0:1, 8, 0:128], recS[:], start=True, stop=True)
            rsb = sbt("rsb", [128, BC])
            nc.vector.tensor_copy(rsb[:], prsb[:])

        # ---- decoder -------------------------------------------------
        with nc.named_scope("l0scale"):
            # c-part scaled per expert (both halves fused; co is input data)
            s0 = sbt("s0", [128, 16, BC])
            for e in range(E):
                nc.vector.tensor_tensor(
                    s0[:, 2 * e : 2 * e + 2, :],
                    co[:, 0:2, :],
                    _bcast_ap(bcastE[:, e, :], 2),
                    AOP.mult,
                )
            for e in range(E):
                nc.vector.tensor_tensor(
                    s0t[32 * (e % 4) : 32 * (e % 4) + 11, e // 4, :],
                    co[0:11, 2, :],
                    bcastE[0:11, e, :],
                    AOP.mult,
                )
        with nc.named_scope("zstack"):
            zs = sbt("zs", [128, 2, BC])
            for e in range(E):
                nc.vector.tensor_tensor(
                    zs[32 * (e % 4) : 32 * (e % 4) + 32, e // 4, :],
                    z[:],
                    bcastE[0:32, e, :],
                    AOP.mult,
                )

        def declayer(name, wt, kfeed, mtiles, nout_f32=None):
            """Run the layer's M-chains into separate psum banks; returns the
            psum tiles. Normalization/elu handled by the caller per half."""
            pls = []
            with nc.named_scope(name):
                for (mlo, mhi) in mtiles:
                    if mhi - mlo == 128:
                        pp = psb()
                    else:
                        pp = ps_sm.tile([mhi - mlo, BC], F32, tag="psm", name="psm")
                    mm_chain(pp[:], kfeed, wt, mlo, mhi)
                    pls.append(pp)
            return pls

        def norm_elu_scale(pp, h, out_act, stack, tagbase):
            """y = psum*1/S -> elu -> out_act[:, h, :]; then per-expert scaled
            copies into stack subtiles h*8+e (None to skip scaling)."""
            y = sbt(tagbase + "_y", [128, BC])
            nc.vector.tensor_mul(y[:], pp[:], rsb[:])
            ex = sbt(tagbase + "_e", [128, BC])
            nc.scalar.activation(ex[:], y[:], AF.Exp)
            tt = sbt(tagbase + "_t", [128, BC])
            nc.vector.tensor_scalar(tt[:], ex[:], 1.0, 1.0, AOP.min, AOP.subtract)
            nc.vector.tensor_tensor(out_act[:, h, :], y[:], tt[:], AOP.max)
            if stack is not None:
                for e in range(E):
                    nc.vector.tensor_tensor(
                        stack[:, h * 8 + e, :],
                        out_act[:, h, :],
                        bcastE[:, e, :],
                        AOP.mult,
                    )

        # L0: K-feed all available up front (c scaled + tails + z + bias)
        l0_feed = [(s0[:, j, :], slice(0, 128)) for j in range(16)]
        l0_feed += [(s0t[:, 0, :], slice(0, 128)), (s0t[:, 1, :], slice(0, 128))]
        l0_feed += [(zs[:, 0, :], slice(0, 128)), (zs[:, 1, :], slice(0, 128)), (expE[:], slice(0, 8))]
        l0o = sbt("l0o", [128, 2, BC])
        s1 = sbt("s1", [128, 16, BC])
        pl0 = declayer("l0", w0, l0_feed, [(0, 128), (128, 256)])
        with nc.named_scope("l0post"):
            norm_elu_scale(pl0[0], 0, l0o, s1, "n0a")
            norm_elu_scale(pl0[1], 1, l0o, s1, "n0b")

        # L1/L2 K-feed: half-0 tiles first, z/bias mid, half-1 tiles last so
        # the chain can start while the previous layer's half-1 post runs.
        def feed12(stack):
            fd = [(stack[:, e, :], slice(0, 128)) for e in range(8)]
            fd += [(zs[:, 0, :], slice(0, 128)), (zs[:, 1, :], slice(0, 128)), (expE[:], slice(0, 8))]
            fd += [(stack[:, 8 + e, :], slice(0, 128)) for e in range(8)]
            return fd

        l1o = sbt("l1o", [128, 2, BC])
        s2 = sbt("s2", [128, 16, BC])
        pl1 = declayer("l1", w1, feed12(s1), [(0, 128), (128, 256)])
        with nc.named_scope("l1post"):
            norm_elu_scale(pl1[0], 0, l1o, s2, "n1a")
            norm_elu_scale(pl1[1], 1, l1o, s2, "n1b")

        pl2 = declayer("l2", w2, feed12(s2), [(0, 128), (128, 256), (256, 267)])
        with nc.named_scope("out"):
            out0 = sbt("out0", [128, 2, BC], F32)
            nc.vector.tensor_mul(out0[:, 0, :], pl2[0][:], rsb[:])
            nc.sync.dma_start(d["outT"][0:128, :], out0[:, 0, :])
            nc.vector.tensor_mul(out0[:, 1, :], pl2[1][:], rsb[:])
            nc.scalar.dma_start(d["outT"][128:256, :], out0[:, 1, :])
            out2 = sbt("out2", [11, BC], F32)
            nc.vector.tensor_mul(out2[:], pl2[2][:], rsb[0:11, :])
            nc.sync.dma_start(d["outT"][256:267, :], out2[:])

    nc.compile()
    return nc


def _host_weights(i):
    """Restack/transpose/cast all weights for the device layout."""
    f16 = np.float16

    def t(a):
        return np.asarray(a, np.float32).T  # [in, out]

    W1t = t(i["fc1_w"])  # [534, 256]
    fc1 = np.concatenate(
        [
            W1t[0:128],
            W1t[128:256],
            _pad_rows(np.concatenate([W1t[256:267], i["fc1_b"][None, :]], 0), 128),
            W1t[267:395],
            W1t[395:523],
            _pad_rows(W1t[523:534], 128),
        ],
        0,
    )
    W2t = t(i["fc2_w"])  # [523, 256]
    fc2 = np.concatenate(
        [
            W2t[0:128],
            W2t[128:256],
            _pad_rows(np.concatenate([W2t[256:267], i["fc2_b"][None, :]], 0), 128),
            W2t[267:395],
            W2t[395:523],
        ],
        0,
    )
    Wmv = np.concatenate([t(i["mu_w"]), t(i["lv_w"])], 1)  # [523, 64]
    bmv = np.concatenate([i["mu_b"], i["lv_b"]])[None, :]
    mv = np.concatenate(
        [
            Wmv[0:128],
            Wmv[128:256],
            _pad_rows(np.concatenate([Wmv[256:267], bmv], 0), 128),
            Wmv[267:395],
            Wmv[395:523],
        ],
        0,
    )
    G0 = t(i["g0_w"])  # [299, 64]
    g0 = np.concatenate(
        [
            G0[32:160],
            G0[160:288],
            _pad_rows(np.concatenate([G0[288:299], i["g0_b"][None, :]], 0), 128),
            _pad_rows(G0[0:32], 128),
        ],
        0,
    )
    # gate elu-fold: [W; -W] (second half multiplies relu(1-exp(x)))
    g1 = np.concatenate([t(i["g1_w"]), -t(i["g1_w"])], 0)
    g2 = np.concatenate([t(i["g2_w"]), -t(i["g2_w"])], 0)

    def dec_stack(w, b, split_prev):
        w = np.asarray(w, np.float32)  # [E, in, out]
        parts = []
        if split_prev:
            # half-0 rows, z-stacks, bias, half-1 rows
            for e in range(E):
                parts.append(w[e, 32:160])
            parts.append(np.concatenate([w[e, 0:32] for e in range(4)], 0))
            parts.append(np.concatenate([w[e, 0:32] for e in range(4, 8)], 0))
            parts.append(_pad_rows(np.asarray(b, np.float32), 128))
            for e in range(E):
                parts.append(w[e, 160:288])
        else:
            for e in range(E):
                parts.append(w[e, 32:160])
                parts.append(w[e, 160:288])
            for g in range(2):
                parts.append(
                    np.concatenate(
                        [_pad_rows(w[e, 288:299], 32) for e in range(4 * g, 4 * g + 4)], 0
                    )
                )
            parts.append(np.concatenate([w[e, 0:32] for e in range(4)], 0))
            parts.append(np.concatenate([w[e, 0:32] for e in range(4, 8)], 0))
            parts.append(_pad_rows(np.asarray(b, np.float32), 128))
        return np.concatenate(parts, 0)

    w0st = dec_stack(i["w0"], i["b0"], False)
    w1st = dec_stack(i["w1"], i["b1"], True)
    w2st = dec_stack(i["w2"], i["b2"], True)

    sel = np.zeros((8, 9, 128), np.float32)
    for e in range(E):
        sel[e, e, :] = 1.0
    sel[:, 8, :] = 1.0

    return {
        "ident": np.eye(128, dtype=np.float32),
        "sel": sel.astype(f16),
        "fc1w": fc1.astype(f16),
        "fc2w": fc2.astype(f16),
        "mvw": mv.astype(f16),
        "g0w": g0.astype(f16),
        "g1w": g1.astype(f16),
        "g1wb": np.asarray(i["g1_b"], np.float32)[None, :].astype(f16),
        "g2w": g2.astype(f16),
        "g2wb": np.asarray(i["g2_b"], np.float32)[None, :].astype(f16),
        "w0st": w0st.astype(f16),
        "w1st": w1st.astype(f16),
        "w2st": w2st.astype(f16),
    }


def kernel(**inputs):
    global LAST_RESULTS
    if "nc" not in _CACHE:
        _CACHE["nc"] = _build_program()
    nc = _CACHE["nc"]

    i = {k: np.asarray(v) for k, v in inputs.items()}
    eps = _eps42()
    wmap = _host_weights(i)

    ones = np.ones((1, B), np.float32)
    xo_full = np.concatenate([np.asarray(i["x"], np.float32).T, ones], 0).astype(np.float16)
    co_full = np.concatenate([np.asarray(i["c"], np.float32).T, ones], 0).astype(np.float16)
    epsT = eps.T.astype(np.float16)

    in_maps = []
    for ci in range(NCORES):
        s = slice(ci * BC, (ci + 1) * BC)
        m = dict(wmap)
        m["xo"] = np.ascontiguousarray(xo_full[:, s])
        m["co"] = np.ascontiguousarray(co_full[:, s])
        m["epsT"] = np.ascontiguousarray(epsT[:, s])
        in_maps.append(m)

    res = run_bass_kernel_spmd(nc, in_maps, core_ids=list(range(NCORES)))
    LAST_RESULTS = res

    out = np.empty((B, F), np.float32)
    mu = np.empty((B, L), np.float32)
    lv = np.empty((B, L), np.float32)
    for ci in range(NCORES):
        s = slice(ci * BC, (ci + 1) * BC)
        r = res.results[ci]
        out[s] = r["outT"].T
        mu[s] = r["mvT"][0:L].T
        lv[s] = r["mvT"][L : 2 * L].T
    return out, mu, lv


# revision 42
# speedup vs baseline: 1.2847x; 1.0418x over previous
"""Trainium2 Bass kernel for CustomPoseMixtureVAE (moe_routing).

Strategy: data-parallel over batch across 8 NeuronCores (256 rows/core),
all weights replicated, no collectives. Activations kept feature-major
[feat, batch] on-chip so every linear is a single PSUM-accumulated GEMM
chain with the (host-pretransposed) weight as the stationary operand;
biases are folded in as extra K-rows against ones/coefficient rows.

The expert mixture  out = einsum('be,bi,eio->bo', coeff, inp, W)  is one
GEMM over K = E*in of per-expert coefficient-scaled inputs stacked along
K. The softmax normalizer stays OFF the critical path: un-normalized
exp(logits) rows are broadcast across partitions with one-hot selector
matmuls and the per-sample 1/sum is multiplied into each layer's psum
output (the mixture is linear in the coefficients).

To keep the PE's HAM clock-gate warm (2.4 GHz) the decoder is scheduled
as one continuous matmul stream: each layer's two M-half accumulation
chains write separate PSUM banks, the half-0 normalize+ELU+rescale runs
on DVE/ACT while the PE is still on half-1, and the next layer's K-feed
is ordered so half-1-dependent tiles come last.

ELU(x) = relu(x) + (min(exp(x),1) - 1)  (exp monotonic); the gate MLP
folds its ELUs into the next matmul via W@elu(x) = W@relu(x) - W'@relu(1
- exp(x)) with W' a host-negated copy.
"""

import numpy as np
from contextlib import ExitStack

import concourse.bass as bass
import concourse.bacc as bacc
import concourse.tile as tile
import concourse.mybir as mybir
from concourse.bass_utils import run_bass_kernel_spmd

F16 = mybir.dt.float16
F32 = mybir.dt.float32
AOP = mybir.AluOpType
AF = mybir.ActivationFunctionType

B = 2048
NCORES = 8
BC = B // NCORES          # 256 batch rows per core
F = 267                   # frame size
L = 32                    # latent
H = 256                   # hidden
E = 8                     # experts
G = 64                    # gate hidden
IN0 = L + F               # 299
IN1 = L + H               # 288

LAST_RESULTS = None       # BassKernelResults of the most recent run
_CACHE = {}


def _eps42():
    """eps = jax.random.normal(key(42), (B, L)) exactly as the reference
    computes it, on the default jax backend (PRNG lowerings differ between
    backends, so we must mirror the reference's code path, not hardcode)."""
    if "eps" not in _CACHE:
        import jax
        import jax.numpy as jnp

        _CACHE["eps"] = np.asarray(
            jax.random.normal(jax.random.key(42), (B, L), jnp.float32)
        )
    return _CACHE["eps"]


def _pad_rows(a, rows):
    out = np.zeros((rows, a.shape[1]), a.dtype)
    out[: a.shape[0]] = a
    return out


def _bcast_ap(ap2d, reps):
    """[P, N] AP -> [P, reps, N] AP with a step-0 middle dim (operand bcast)."""
    return bass.AP(
        tensor=ap2d.tensor,
        offset=ap2d.offset,
        ap=[ap2d.ap[0], [0, reps], ap2d.ap[1]],
    )


def _build_program():
    nc = bacc.Bacc("TRN2")
    d = {}

    def din(name, shape, dt=F16):
        d[name] = nc.dram_tensor(name, shape, dt, kind="ExternalInput").ap()

    def dout(name, shape, dt=F32):
        d[name] = nc.dram_tensor(name, shape, dt, kind="ExternalOutput").ap()

    # per-core activations ([feat, batch] fp16; xo/co carry a trailing ones row)
    din("xo", [268, BC])
    din("co", [268, BC])
    din("epsT", [L, BC])
    # replicated weights (host-restacked, see kernel())
    din("sel", [8, 9, 128])
    din("ident", [128, 128], F32)
    din("fc1w", [6 * 128, 256])
    din("fc2w", [5 * 128, 256])
    din("mvw", [5 * 128, 64])
    din("g0w", [4 * 128, 64])
    din("g1w", [128, 64])
    din("g1wb", [1, 64])
    din("g2w", [128, 8])
    din("g2wb", [1, 8])
    din("w0st", [21 * 128, 256])
    din("w1st", [19 * 128, 256])
    din("w2st", [19 * 128, 267])
    dout("outT", [F, BC])
    dout("mvT", [2 * L, BC])

    with tile.TileContext(nc) as tc, ExitStack() as ctx:
        sb = ctx.enter_context(tc.tile_pool(name="sb", bufs=1))
        # separate banks per M-half so DVE can read half-0 while the PE
        # accumulates half-1 (same-bank PE-W/DVE-R would serialize)
        ps_big = ctx.enter_context(tc.tile_pool(name="ps_big", bufs=4, space="PSUM"))
        ps_sm = ctx.enter_context(tc.tile_pool(name="ps_sm", bufs=2, space="PSUM"))
        ps_bc = ctx.enter_context(tc.tile_pool(name="ps_bc", bufs=2, space="PSUM"))

        def sbt(tag, shape, dt=F16):
            return sb.tile(shape, dt, tag=tag, name=tag)

        def psb():
            return ps_big.tile([128, BC], F32, tag="pbig", name="pbig")

        # ---- input / weight DMAs -------------------------------------
        # DMA issue occupies the issuing engine's FIFO (~650ns per
        # dma_start): scalar only gets what it must, bulk goes on sync,
        # tiny constants on the gpsimd SWDGE.
        def dma_tiled(dst, src, nk, cols, eng):
            eng.dma_start(
                dst[:, 0:nk, :],
                src[0 : nk * 128, :].rearrange("(k p) m -> p k m", p=128),
            )

        xo = sbt("xo", [128, 3, BC])
        nc.sync.dma_start(xo[:, 0, :].rearrange("p n -> p () n"), d["xo"][0:128, :].rearrange("(k p) n -> p k n", p=128))
        fc1w = sbt("fc1w", [128, 6, 256])
        nc.sync.dma_start(fc1w[:, 0:1, :], d["fc1w"][0:128, :].rearrange("(k p) m -> p k m", p=128))
        nc.sync.dma_start(xo[:, 1, :].rearrange("p n -> p () n"), d["xo"][128:256, :].rearrange("(k p) n -> p k n", p=128))
        nc.sync.dma_start(fc1w[:, 1:3, :], d["fc1w"][128:384, :].rearrange("(k p) m -> p k m", p=128))
        co = sbt("co", [128, 3, BC])
        nc.scalar.dma_start(co[:, 0:2, :], d["co"][0:256, :].rearrange("(k p) n -> p k n", p=128))
        nc.scalar.dma_start(fc1w[:, 3:6, :], d["fc1w"][384:768, :].rearrange("(k p) m -> p k m", p=128))
        nc.gpsimd.dma_start(xo[0:12, 2, :], d["xo"][256:268, :])
        nc.gpsimd.dma_start(co[0:12, 2, :], d["co"][256:268, :])
        fc2w = sbt("fc2w", [128, 5, 256])
        nc.sync.dma_start(fc2w[:, 0:3, :], d["fc2w"][0:384, :].rearrange("(k p) m -> p k m", p=128))
        nc.scalar.dma_start(fc2w[:, 3:5, :], d["fc2w"][384:640, :].rearrange("(k p) m -> p k m", p=128))
        eps = sbt("eps", [L, BC])
        nc.gpsimd.dma_start(eps[:], d["epsT"][:])
        sel = sbt("sel", [8, 9, 128])
        nc.gpsimd.dma_start(sel[:], d["sel"][:])
        ident = sbt("ident", [128, 128], F32)
        nc.gpsimd.dma_start(ident[:], d["ident"][:])

        # touch the exp table now so ACT_TABLE_LOAD overlaps the input DMAs
        scratch1 = sbt("scratch1", [1, 8])
        nc.scalar.activation(scratch1[:], sel[0:1, 8, 0:8], AF.Exp)

        mvw = sbt("mvw", [128, 5, 64])
        dma_tiled(mvw, d["mvw"], 5, 64, nc.gpsimd)
        g0w = sbt("g0w", [128, 4, 64])
        dma_tiled(g0w, d["g0w"], 4, 64, nc.gpsimd)
        g1w = sbt("g1w", [128, 64])
        nc.gpsimd.dma_start(g1w[:], d["g1w"][:])
        g1wb = sbt("g1wb", [1, 64])
        nc.gpsimd.dma_start(g1wb[:], d["g1wb"][:])
        g2w = sbt("g2w", [128, 8])
        nc.gpsimd.dma_start(g2w[:], d["g2w"][:])
        g2wb = sbt("g2wb", [1, 8])
        nc.gpsimd.dma_start(g2wb[:], d["g2wb"][:])

        w0 = sbt("w0", [128, 21, 256])
        w1 = sbt("w1", [128, 19, 256])
        w2 = sbt("w2", [128, 19, 267])
        for (wt, src, nk) in ((w0, d["w0st"], 21), (w1, d["w1st"], 19), (w2, d["w2st"], 19)):
            mid = (nk + 1) // 2
            for lo, hi in ((0, mid), (mid, nk)):
                nc.sync.dma_start(
                    wt[:, lo:hi, :],
                    src[lo * 128 : hi * 128, :].rearrange("(k p) m -> p k m", p=128),
                )

        # L0 c-tail scaled tiles: zero the 21-row pads once, up front
        s0t = sbt("s0t", [128, 2, BC])
        nc.vector.memset(s0t[:], 0.0)

        # ---- helpers -------------------------------------------------
        def mm_chain(pap, kfeed, wt, mlo, mhi):
            n = len(kfeed)
            for i, (rh, krows) in enumerate(kfeed):
                nc.tensor.matmul(
                    pap, wt[krows, i, mlo:mhi], rh, start=(i == 0), stop=(i == n - 1)
                )

        def elu_half(pp, out_ap, tagbase):
            """ELU from psum half pp [P, BC] into fp16 out_ap."""
            p = pp.shape[0]
            ex = sbt(tagbase + "_e", [p, BC])
            nc.scalar.activation(ex[:], pp, AF.Exp)
            tt = sbt(tagbase + "_t", [p, BC])
            nc.vector.tensor_scalar(tt[:], ex[:], 1.0, 1.0, AOP.min, AOP.subtract)
            nc.vector.scalar_tensor_tensor(out_ap, pp, 0.0, tt[:], AOP.max, AOP.add)

        # ---- encoder -------------------------------------------------
        enc1_rhs = [
            (xo[:, 0, :], slice(0, 128)),
            (xo[:, 1, :], slice(0, 128)),
            (xo[0:12, 2, :], slice(0, 12)),
            (co[:, 0, :], slice(0, 128)),
            (co[:, 1, :], slice(0, 128)),
            (co[0:12, 2, :], slice(0, 12)),
        ]
        h1 = sbt("h1", [128, 2, BC])
        with nc.named_scope("enc1"):
            for m in range(2):
                pp = psb()
                mm_chain(pp[:], enc1_rhs, fc1w, m * 128, (m + 1) * 128)
                elu_half(pp[:], h1[:, m, :], f"eh1{m}")

        enc2_rhs = [
            (xo[:, 0, :], slice(0, 128)),
            (xo[:, 1, :], slice(0, 128)),
            (xo[0:12, 2, :], slice(0, 12)),
            (h1[:, 0, :], slice(0, 128)),
            (h1[:, 1, :], slice(0, 128)),
        ]
        h2 = sbt("h2", [128, 2, BC])
        with nc.named_scope("enc2"):
            for m in range(2):
                pp = psb()
                mm_chain(pp[:], enc2_rhs, fc2w, m * 128, (m + 1) * 128)
                elu_half(pp[:], h2[:, m, :], f"eh2{m}")

        with nc.named_scope("muvar"):
            mv_rhs = [
                (xo[:, 0, :], slice(0, 128)),
                (xo[:, 1, :], slice(0, 128)),
                (xo[0:12, 2, :], slice(0, 12)),
                (h2[:, 0, :], slice(0, 128)),
                (h2[:, 1, :], slice(0, 128)),
            ]
            pmv = ps_sm.tile([64, BC], F32, tag="psm", name="psm")
            mm_chain(pmv[:], mv_rhs, mvw, 0, 64)
            # z = mu + eps * exp(0.5*logvar)
            stdt = sbt("stdt", [L, BC])
            nc.scalar.activation(stdt[:], pmv[32:64, :], AF.Exp, scale=0.5)
            zt = sbt("zt", [L, BC])
            nc.vector.tensor_mul(zt[:], eps[:], stdt[:])
            z = sbt("z", [L, BC])
            nc.vector.tensor_add(z[:], zt[:], pmv[0:32, :])
            mvf = sbt("mvf", [64, BC], F32)
            nc.scalar.copy(mvf[:], pmv[:])
            nc.sync.dma_start(d["mvT"][:], mvf[:])

        # ---- gate ----------------------------------------------------
        # ELUs folded into the next matmul: W@elu(x) = W@relu(x) -
        # Wneg@relu(1-exp(x)), both halves stacked into one 128-row K-tile.
        with nc.named_scope("gate"):
            onesr = sbt("onesr", [1, BC])
            nc.vector.memset(onesr[:], 1.0)
            pg0 = ps_sm.tile([64, BC], F32, tag="psm", name="psm")
            g0_rhs = [
                (co[:, 0, :], slice(0, 128)),
                (co[:, 1, :], slice(0, 128)),
                (co[0:12, 2, :], slice(0, 12)),
                (z[:], slice(0, 32)),
            ]
            mm_chain(pg0[:], g0_rhs, g0w, 0, 64)

            def gate_relu_t(pp, tagbase):
                ga = sbt(tagbase + "_a", [128, BC])
                ex = sbt(tagbase + "_e", [64, BC])
                nc.scalar.activation(ex[:], pp, AF.Exp)
                nc.vector.tensor_scalar_max(ga[0:64, :], pp, 0.0)
                # relu(1 - exp(x)) = -(min(exp(x),1) - 1); sign absorbed by Wneg
                nc.scalar.activation(ga[64:128, :], ex[:], AF.Relu, bias=1.0, scale=-1.0)
                return ga

            g1in = gate_relu_t(pg0[:], "eg0")
            pg1 = ps_sm.tile([64, BC], F32, tag="psm", name="psm")
            nc.tensor.matmul(pg1[:], g1w[:, 0:64], g1in[:], start=True, stop=False)
            nc.tensor.matmul(pg1[:], g1wb[0:1, 0:64], onesr[:], start=False, stop=True)

            g2in = gate_relu_t(pg1[:], "eg1")
            plg = ps_sm.tile([8, BC], F32, tag="psm", name="psm")
            nc.tensor.matmul(plg[:], g2w[:, 0:8], g2in[:], start=True, stop=False)
            nc.tensor.matmul(plg[:], g2wb[0:1, 0:8], onesr[:], start=False, stop=True)

            # un-normalized softmax numerators, broadcast per expert
            expE = sbt("expE", [8, BC])
            nc.scalar.activation(expE[:], plg[:], AF.Exp)

            bcastE = sbt("bcastE", [128, 8, BC])
            for i in range(4):
                pbc = ps_bc.tile([128, 512], F32, tag="pbc", name="pbc")
                nc.tensor.matmul(pbc[:, 0:BC], sel[:, 2 * i, :], expE[:], start=True, stop=True)
                nc.tensor.matmul(pbc[:, BC:512], sel[:, 2 * i + 1, :], expE[:], start=True, stop=True)
                dst = bcastE[:, 2 * i : 2 * i + 2, :].rearrange("p k n -> p (k n)")
                nc.scalar.copy(dst, pbc[:])

            # 1/sum pipeline (fully off the critical path): sum with batch on
            # partitions so the DVE divide runs 2 elem/lane, transpose back
            # via identity matmuls, broadcast to 128 rows.
            pst = ps_sm.tile([128, 2], F32, tag="psm", name="psm")
            nc.tensor.matmul(pst[:, 0:1], expE[0:8, 0:128], sel[0:8, 8, 0:1], start=True, stop=True)
            nc.tensor.matmul(pst[:, 1:2], expE[0:8, 128:256], sel[0:8, 8, 0:1], start=True, stop=True)
            rst = sbt("rst", [128, 2], F32)
            with nc.allow_low_precision(reason="softmax denominators are well-conditioned"):
                nc.vector.reciprocal(rst[:], pst[:])
            precS = ps_sm.tile([1, BC], F32, tag="psm", name="psm")
            nc.tensor.matmul(precS[0:1, 0:128], rst[:, 0:1], ident[:], start=True, stop=True)
            nc.tensor.matmul(precS[0:1, 128:256], rst[:, 1:2], ident[:], start=True, stop=True)
            recS = sbt("recS", [1, BC])
            nc.scalar.copy(recS[:], precS[:])
            prsb = ps_sm.tile([128, BC], F32, tag="psm", name="psm")
            nc.tensor.matmul(prsb[:], sel[0:1, 8, 0:128], recS[:], start=True, stop=True)
            rsb = sbt("rsb", [128, BC])
            nc.scalar.copy(rsb[:], prsb[:])

        # ---- decoder -------------------------------------------------
        with nc.named_scope("l0scale"):
            # c-part scaled per expert (both halves fused; co is input data)
            s0 = sbt("s0", [128, 16, BC])
            for e in range(E):
                nc.vector.tensor_tensor(
                    s0[:, 2 * e : 2 * e + 2, :],
                    co[:, 0:2, :],
                    _bcast_ap(bcastE[:, e, :], 2),
                    AOP.mult,
                )
            for e in range(E):
                nc.vector.tensor_tensor(
                    s0t[32 * (e % 4) : 32 * (e % 4) + 11, e // 4, :],
                    co[0:11, 2, :],
                    bcastE[0:11, e, :],
                    AOP.mult,
                )
        with nc.named_scope("zstack"):
            zs = sbt("zs", [128, 2, BC])
            for e in range(E):
                nc.vector.tensor_tensor(
                    zs[32 * (e % 4) : 32 * (e % 4) + 32, e // 4, :],
                    z[:],
                    bcastE[0:32, e, :],
                    AOP.mult,
                )

        def declayer(name, wt, kfeed, mtiles, nout_f32=None):
            """Run the layer's M-chains into separate psum banks; returns the
            psum tiles. Normalization/elu handled by the caller per half."""
            pls = []
            with nc.named_scope(name):
                for (mlo, mhi) in mtiles:
                    if mhi - mlo == 128:
                        pp = psb()
                    else:
                        pp = ps_sm.tile([mhi - mlo, BC], F32, tag="psm", name="psm")
                    mm_chain(pp[:], kfeed, wt, mlo, mhi)
                    pls.append(pp)
            return pls

        def norm_elu_scale(pp, h, out_act, stack, tagbase):
            """y = psum*1/S -> elu -> out_act[:, h, :]; then per-expert scaled
            copies into stack subtiles h*8+e (None to skip scaling)."""
            y = sbt(tagbase + "_y", [128, BC])
            nc.vector.tensor_mul(y[:], pp[:], rsb[:])
            ex = sbt(tagbase + "_e", [128, BC])
            nc.scalar.activation(ex[:], y[:], AF.Exp)
            tt = sbt(tagbase + "_t", [128, BC])
            nc.vector.tensor_scalar(tt[:], ex[:], 1.0, 1.0, AOP.min, AOP.subtract)
            nc.vector.tensor_tensor(out_act[:, h, :], y[:], tt[:], AOP.max)
            if stack is not None:
                for e in range(E):
                    nc.vector.tensor_tensor(
                        stack[:, h * 8 + e, :],
                        out_act[:, h, :],
                        bcastE[:, e, :],
                        AOP.mult,
                    )

        # L0: K-feed all available up front (c scaled + tails + z + bias)
        l0_feed = [(s0[:, j, :], slice(0, 128)) for j in range(16)]
        l0_feed += [(s0t[:, 0, :], slice(0, 128)), (s0t[:, 1, :], slice(0, 128))]
        l0_feed += [(zs[:, 0, :], slice(0, 128)), (zs[:, 1, :], slice(0, 128)), (expE[:], slice(0, 8))]
        l0o = sbt("l0o", [128, 2, BC])
        s1 = sbt("s1", [128, 16, BC])
        pl0 = declayer("l0", w0, l0_feed, [(0, 128), (128, 256)])
        with nc.named_scope("l0post"):
            norm_elu_scale(pl0[0], 0, l0o, s1, "n0a")
            norm_elu_scale(pl0[1], 1, l0o, s1, "n0b")

        # L1/L2 K-feed: half-0 tiles first, z/bias mid, half-1 tiles last so
        # the chain can start while the previous layer's half-1 post runs.
        def feed12(stack):
            fd = [(stack[:, e, :], slice(0, 128)) for e in range(8)]
            fd += [(zs[:, 0, :], slice(0, 128)), (zs[:, 1, :], slice(0, 128)), (expE[:], slice(0, 8))]
            fd += [(stack[:, 8 + e, :], slice(0, 128)) for e in range(8)]
            return fd

        l1o = sbt("l1o", [128, 2, BC])
        s2 = sbt("s2", [128, 16, BC])
        pl1 = declayer("l1", w1, feed12(s1), [(0, 128), (128, 256)])
        with nc.named_scope("l1post"):
            norm_elu_scale(pl1[0], 0, l1o, s2, "n1a")
            norm_elu_scale(pl1[1], 1, l1o, s2, "n1b")

        pl2 = declayer("l2", w2, feed12(s2), [(0, 128), (128, 256), (256, 267)])
        with nc.named_scope("out"):
            out0 = sbt("out0", [128, 2, BC], F32)
            nc.vector.tensor_mul(out0[:, 0, :], pl2[0][:], rsb[:])
            nc.sync.dma_start(d["outT"][0:128, :], out0[:, 0, :])
            nc.vector.tensor_mul(out0[:, 1, :], pl2[1][:], rsb[:])
            nc.scalar.dma_start(d["outT"][128:256, :], out0[:, 1, :])
            out2 = sbt("out2", [11, BC], F32)
            nc.vector.tensor_mul(out2[:], pl2[2][:], rsb[0:11, :])
            nc.sync.dma_start(d["outT"][256:267, :], out2[:])

    nc.compile()
    return nc


def _host_weights(i):
    """Restack/transpose/cast all weights for the device layout."""
    f16 = np.float16

    def t(a):
        return np.asarray(a, np.float32).T  # [in, out]

    W1t = t(i["fc1_w"])  # [534, 256]
    fc1 = np.concatenate(
        [
            W1t[0:128],
            W1t[128:256],
            _pad_rows(np.concatenate([W1t[256:267], i["fc1_b"][None, :]], 0), 128),
            W1t[267:395],
            W1t[395:523],
            _pad_rows(W1t[523:534], 128),
        ],
        0,
    )
    W2t = t(i["fc2_w"])  # [523, 256]
    fc2 = np.concatenate(
        [
            W2t[0:128],
            W2t[128:256],
            _pad_rows(np.concatenate([W2t[256:267], i["fc2_b"][None, :]], 0), 128),
            W2t[267:395],
            W2t[395:523],
        ],
        0,
    )
    Wmv = np.concatenate([t(i["mu_w"]), t(i["lv_w"])], 1)  # [523, 64]
    bmv = np.concatenate([i["mu_b"], i["lv_b"]])[None, :]
    mv = np.concatenate(
        [
            Wmv[0:128],
            Wmv[128:256],
            _pad_rows(np.concatenate([Wmv[256:267], bmv], 0), 128),
            Wmv[267:395],
            Wmv[395:523],
        ],
        0,
    )
    G0 = t(i["g0_w"])  # [299, 64]
    g0 = np.concatenate(
        [
            G0[32:160],
            G0[160:288],
            _pad_rows(np.concatenate([G0[288:299], i["g0_b"][None, :]], 0), 128),
            _pad_rows(G0[0:32], 128),
        ],
        0,
    )
    # gate elu-fold: [W; -W] (second half multiplies relu(1-exp(x)))
    g1 = np.concatenate([t(i["g1_w"]), -t(i["g1_w"])], 0)
    g2 = np.concatenate([t(i["g2_w"]), -t(i["g2_w"])], 0)

    def dec_stack(w, b, split_prev):
        w = np.asarray(w, np.float32)  # [E, in, out]
        parts = []
        if split_prev:
            # half-0 rows, z-stacks, bias, half-1 rows
            for e in range(E):
                parts.append(w[e, 32:160])
            parts.append(np.concatenate([w[e, 0:32] for e in range(4)], 0))
            parts.append(np.concatenate([w[e, 0:32] for e in range(4, 8)], 0))
            parts.append(_pad_rows(np.asarray(b, np.float32), 128))
            for e in range(E):
                parts.append(w[e, 160:288])
        else:
            for e in range(E):
                parts.append(w[e, 32:160])
                parts.append(w[e, 160:288])
            for g in range(2):
                parts.append(
                    np.concatenate(
                        [_pad_rows(w[e, 288:299], 32) for e in range(4 * g, 4 * g + 4)], 0
                    )
                )
            parts.append(np.concatenate([w[e, 0:32] for e in range(4)], 0))
            parts.append(np.concatenate([w[e, 0:32] for e in range(4, 8)], 0))
            parts.append(_pad_rows(np.asarray(b, np.float32), 128))
        return np.concatenate(parts, 0)

    w0st = dec_stack(i["w0"], i["b0"], False)
    w1st = dec_stack(i["w1"], i["b1"], True)
    w2st = dec_stack(i["w2"], i["b2"], True)

    sel = np.zeros((8, 9, 128), np.float32)
    for e in range(E):
        sel[e, e, :] = 1.0
    sel[:, 8, :] = 1.0

    return {
        "ident": np.eye(128, dtype=np.float32),
        "sel": sel.astype(f16),
        "fc1w": fc1.astype(f16),
        "fc2w": fc2.astype(f16),
        "mvw": mv.astype(f16),
        "g0w": g0.astype(f16),
        "g1w": g1.astype(f16),
        "g1wb": np.asarray(i["g1_b"], np.float32)[None, :].astype(f16),
        "g2w": g2.astype(f16),
        "g2wb": np.asarray(i["g2_b"], np.float32)[None, :].astype(f16),
        "w0st": w0st.astype(f16),
        "w1st": w1st.astype(f16),
        "w2st": w2st.astype(f16),
    }


def kernel(**inputs):
    global LAST_RESULTS
    if "nc" not in _CACHE:
        _CACHE["nc"] = _build_program()
    nc = _CACHE["nc"]

    i = {k: np.asarray(v) for k, v in inputs.items()}
    eps = _eps42()
    wmap = _host_weights(i)

    ones = np.ones((1, B), np.float32)
    xo_full = np.concatenate([np.asarray(i["x"], np.float32).T, ones], 0).astype(np.float16)
    co_full = np.concatenate([np.asarray(i["c"], np.float32).T, ones], 0).astype(np.float16)
    epsT = eps.T.astype(np.float16)

    in_maps = []
    for ci in range(NCORES):
        s = slice(ci * BC, (ci + 1) * BC)
        m = dict(wmap)
        m["xo"] = np.ascontiguousarray(xo_full[:, s])
        m["co"] = np.ascontiguousarray(co_full[:, s])
        m["epsT"] = np.ascontiguousarray(epsT[:, s])
        in_maps.append(m)

    res = run_bass_kernel_spmd(nc, in_maps, core_ids=list(range(NCORES)))
    LAST_RESULTS = res

    out = np.empty((B, F), np.float32)
    mu = np.empty((B, L), np.float32)
    lv = np.empty((B, L), np.float32)
    for ci in range(NCORES):
        s = slice(ci * BC, (ci + 1) * BC)
        r = res.results[ci]
        out[s] = r["outT"].T
        mu[s] = r["mvT"][0:L].T
        lv[s] = r["mvT"][L : 2 * L].T
    return out, mu, lv
